# revision 4
# baseline (speedup 1.0000x reference)
"""Trainium2 Bass kernel for nn_AdvancedFastMQA — v2.

Data-parallel over batch B=8 across 8 NeuronCores. Transposed dataflow
(no on-device transposes except V). Per-core PE-cycle cuts vs v1:

 - Sliding-window overlap sharing: score tiles and attention@V partial
   sums for the k-chunk regions shared by adjacent windows are computed
   once (40 unit tiles instead of 52 for both scores and AV). Window
   outputs are assembled from 2-chunk PSUM partials (A,B,C1,C2,D,F) with
   cheap DVE combines.
 - Denominator: instead of M=1 ones-vector matmuls per k-chunk (same
   streaming cost as scores, zero useful flops), the k-chunk sigmoid
   tiles are summed on DVE and a single ones[128x128] matmul per window
   produces the partition-sum already broadcast across 128 partitions
   (also kills the gpsimd broadcast).
 - O-projection computed transposed: yT[o,t] = sum_i wo_tile[i].T @ ao_i
   with the weight stationary for 2 matmuls each, PSUM double-buffered;
   host transposes the [4096,1024] result back.
 - Q-projection of head h+1 is interleaved into attention of head h so
   the PE never waits on ACT sigmoids.

Windows (S=1024, window=512, stride 256):
  w0: k,q in [0,512); w1: k,q [256,768); w2: k,q [512,1024);
  w3: k,q [768,1024). Blend regions: [256,512) w0/w1, [512,768) w1/w2,
  [768,1024) w2/w3, alpha = linspace(0,1,256).

AV partial-sum plan (per head, PSUM tiles, kc = 128-wide k chunks):
  A  = kc0+kc1 over q[0:512)      B  = kc2+kc3 over q[0:512)
  C1 = kc4+kc5 over q[256:768)    C2 = kc2+kc3 over q[512:768)
  D  = kc6+kc7 over q[512:1024)   F  = kc4+kc5 over q[768:1024)
  u0 = A+B; u1 = B[256:512)+C1 | C2+C1[512:768); u2 = C1+D | D+F;
  u3 = D[768:1024).
"""

import sys

for _p in ("/opt/trn_rl_repo", "/opt/pypackages"):
    if _p not in sys.path:
        sys.path.append(_p)

import numpy as np
import ml_dtypes

import concourse.bacc as bacc
import concourse.tile as tile
import concourse.mybir as mybir
import concourse.bass_isa as bass_isa
from concourse.bass_utils import run_bass_kernel_spmd

BF16 = mybir.dt.bfloat16
F32 = mybir.dt.float32
AF = mybir.ActivationFunctionType

B, S, HD = 8, 1024, 4096
H, DH = 32, 128
WINDOW = 512
SCALE = 1.0 / float(np.sqrt(DH))
ROPE_BASE = 10000.0
NI = HD // 128          # 32 contraction chunks
NT = S // 128           # 8 token chunks

# a-tile (sigmoid) layout: per kc the union of q-ranges that need it.
A_QLO = [0, 0, 0, 0, 256, 256, 512, 512]
A_W = [512, 512, 768, 768, 768, 768, 512, 512]
A_OFF = [0, 512, 1024, 1792, 2560, 3328, 4096, 4608]
A_TOT = 5120

_CACHE = {}


def _rope_cache_np(S_, D_, base=ROPE_BASE):
    inv_freq = 1.0 / (base ** (np.arange(0, D_, 2, dtype=np.float32) / D_))
    t = np.arange(S_, dtype=np.float32)
    f = np.outer(t, inv_freq)
    cos = np.zeros((S_, D_), dtype=np.float32)
    sin = np.zeros((S_, D_), dtype=np.float32)
    cos[:, 0::2] = np.cos(f)
    cos[:, 1::2] = np.cos(f)
    sin[:, 0::2] = np.sin(f)
    sin[:, 1::2] = np.sin(f)
    return cos, sin


def build_nc():
    nc = bacc.Bacc("TRN2", debug=False, target_bir_lowering=False)

    xT_d = nc.dram_tensor("xT", [HD, S], BF16, kind="ExternalInput").ap()
    wq_d = nc.dram_tensor("wq", [H, 128, HD], BF16, kind="ExternalInput").ap()
    wk_d = nc.dram_tensor("wk", [128, HD], BF16, kind="ExternalInput").ap()
    wv_d = nc.dram_tensor("wv", [128, HD], BF16, kind="ExternalInput").ap()
    wo_d = nc.dram_tensor("wo", [32, 128, HD], BF16, kind="ExternalInput").ap()
    cos_d = nc.dram_tensor("cosT", [128, S], BF16, kind="ExternalInput").ap()
    sin_d = nc.dram_tensor("sinS", [128, S], BF16, kind="ExternalInput").ap()
    alpha_d = nc.dram_tensor("alphaB", [128, 256], BF16, kind="ExternalInput").ap()
    rotm_d = nc.dram_tensor("rotm", [128, 128], BF16, kind="ExternalInput").ap()
    iden_d = nc.dram_tensor("ident", [128, 128], BF16, kind="ExternalInput").ap()
    y_d = nc.dram_tensor("y", [HD, S], F32, kind="ExternalOutput").ap()

    with tile.TileContext(nc) as tc:
        with tc.tile_pool(name="consts", bufs=1) as cp:
            xt = cp.tile([128, NI * S], BF16)              # 64KB/part
            cos_t = cp.tile([128, S], BF16)
            sin_t = cp.tile([128, S], BF16)
            alpha_t = cp.tile([128, 256], BF16)
            ones_t = cp.tile([128, 128], BF16)
            nc.vector.memset(ones_t[:], 1.0)
            rotm_t = cp.tile([128, 128], BF16)
            iden_t = cp.tile([128, 128], BF16)

            kr_t = cp.tile([128, S], BF16)                 # roped K
            v_all = cp.tile([128, NT * 128], BF16)         # V as 8 lhsT tiles
            ao = cp.tile([128, H * S], BF16)               # attention out, 64KB/part

            with tc.tile_pool(name="work", bufs=1) as wp:
              with tc.tile_pool(name="ps", bufs=1, space="PSUM") as pp:

                def rope_mc(src):
                    mc = wp.tile([128, S], BF16, tag="rope_mc", bufs=1)
                    nc.vector.tensor_mul(mc[:], src[:], cos_t[:])
                    return mc

                def rope_rot(dst, src, mc, tag):
                    for rh in range(2):
                        rp = pp.tile([128, 512], F32, tag="pden", bufs=1,
                                     name=f"rot_{tag}_{rh}")
                        nc.tensor.matmul(
                            rp[:], lhsT=rotm_t[:],
                            rhs=src[:, rh * 512:(rh + 1) * 512],
                            start=True, stop=True,
                        )
                        ms = wp.tile([128, 512], BF16, tag="rope_ms", bufs=1)
                        nc.vector.tensor_mul(ms[:], rp[:], sin_t[:, rh * 512:(rh + 1) * 512])
                        nc.vector.tensor_add(
                            dst[:, rh * 512:(rh + 1) * 512],
                            mc[:, rh * 512:(rh + 1) * 512], ms[:],
                        )

                def rope(dst, src, tag):
                    # dst = src*cos + rotate_half(src)*sin; rotate via PE
                    rope_rot(dst, src, rope_mc(src), tag)

                # ---- phase 1: interleaved K / VT / Q-head0 projections ----
                # DMA order matters: the first matmuls need the weights and
                # x chunk 0, so those go first; bulk x and rope consts after.
                wk_t = wp.tile([128, HD], BF16, tag="wq", bufs=3, name="wk")
                wv_t = wp.tile([128, HD], BF16, tag="wq", bufs=3, name="wv")
                wq0_t = wp.tile([128, HD], BF16, tag="wq", bufs=3, name="wq0")
                def wpiece(p):
                    c0 = p * 1024
                    nc.sync.dma_start(out=wk_t[:, c0:c0 + 1024],
                                      in_=wk_d[:, c0:c0 + 1024])
                    nc.sync.dma_start(out=wv_t[:, c0:c0 + 1024],
                                      in_=wv_d[:, c0:c0 + 1024])
                    nc.sync.dma_start(out=wq0_t[:, c0:c0 + 1024],
                                      in_=wq_d[0, :, c0:c0 + 1024])

                # DMA packets drain the queue ~in order, so emit transfers in
                # exact consumption order: weight piece p just before the x
                # chunks that use it.
                wpiece(0)
                for i in range(NI):
                    nc.sync.dma_start(
                        out=xt[:, i * S:(i + 1) * S], in_=xT_d[i * 128:(i + 1) * 128, :]
                    )
                    if i == 5:
                        wpiece(1)
                    if i == 13:
                        wpiece(2)
                    if i == 21:
                        wpiece(3)
                    if i == 8:
                        nc.sync.dma_start(out=cos_t[:], in_=cos_d[:])
                        nc.sync.dma_start(out=sin_t[:], in_=sin_d[:])
                    if i == 10:
                        nc.sync.dma_start(out=alpha_t[:], in_=alpha_d[:])
                        nc.sync.dma_start(out=rotm_t[:], in_=rotm_d[:])
                        nc.sync.dma_start(out=iden_t[:], in_=iden_d[:])
                kps = [pp.tile([128, 512], F32, tag="pscore", bufs=2, name=f"kp{hh}") for hh in range(2)]
                vps = [pp.tile([128, 512], F32, tag="pav", bufs=3, name=f"vp{hh}") for hh in range(2)]
                qps0 = [pp.tile([128, 512], F32, tag="pproj", bufs=2, name=f"qp0{hh}") for hh in range(2)]
                for i in range(NI):
                    st_ = (i == 0)
                    sp_ = (i == NI - 1)
                    for hh in range(2):
                        rhs = xt[:, i * S + hh * 512: i * S + (hh + 1) * 512]
                        nc.tensor.matmul(kps[hh][:], lhsT=wk_t[:, i * 128:(i + 1) * 128],
                                         rhs=rhs, start=st_, stop=sp_)
                        nc.tensor.matmul(vps[hh][:], lhsT=wv_t[:, i * 128:(i + 1) * 128],
                                         rhs=rhs, start=st_, stop=sp_)
                        nc.tensor.matmul(qps0[hh][:], lhsT=wq0_t[:, i * 128:(i + 1) * 128],
                                         rhs=rhs, start=st_, stop=sp_)
                kraw = wp.tile([128, S], BF16, tag="kraw", bufs=1)
                vtraw = wp.tile([128, S], BF16, tag="qrt", bufs=2)
                qraw0 = wp.tile([128, S], BF16, tag="qraw", bufs=1)
                for hh in range(2):
                    nc.scalar.copy(kraw[:, hh * 512:(hh + 1) * 512], kps[hh][:])
                    nc.scalar.copy(vtraw[:, hh * 512:(hh + 1) * 512], vps[hh][:])
                    nc.scalar.copy(qraw0[:, hh * 512:(hh + 1) * 512], qps0[hh][:])
                rope(kr_t, kraw, "k")
                for t in range(NT):
                    tp = pp.tile([128, 128], BF16, tag="pscore", bufs=2, name=f"vtp{t}")
                    nc.tensor.transpose(tp[:], vtraw[:, t * 128:(t + 1) * 128], iden_t[:])
                    nc.scalar.copy(v_all[:, t * 128:(t + 1) * 128], tp[:])
                qrt0 = wp.tile([128, S], BF16, tag="qrt", bufs=2)
                rope(qrt0, qraw0, "q0")

                # ---- phase 2: per-head attention + interleaved Q proj(h+1) ----
                def acol(kc, q):
                    return A_OFF[kc] + q - A_QLO[kc]

                qrt_holder = [qrt0]
                wot_pre = {}
                yps0 = None
                for h in range(H):
                    qrt = qrt_holder[0]

                    # Q projection emitters for head h+1, in 8 groups of 4 i's
                    if h + 1 < H:
                        wq_t = wp.tile([128, HD], BF16, tag="wq", bufs=3)
                        nc.sync.dma_start(out=wq_t[:], in_=wq_d[h + 1])
                        qps = [pp.tile([128, 512], F32, tag="pproj", bufs=2,
                                       name=f"qp{h+1}_{hh}") for hh in range(2)]

                        QG = [(0, 5), (5, 10), (10, 15), (15, 20), (20, 26),
                              (26, 32)]

                        def qgroup(g, wq_t=wq_t, qps=qps):
                            for i in range(*QG[g]):
                                for hh in range(2):
                                    nc.tensor.matmul(
                                        qps[hh][:],
                                        lhsT=wq_t[:, i * 128:(i + 1) * 128],
                                        rhs=xt[:, i * S + hh * 512: i * S + (hh + 1) * 512],
                                        start=(i == 0), stop=(i == NI - 1),
                                    )
                    else:
                        # last head: no Q projection to interleave — fill the
                        # PE with O-proj tile 0 (heads 0..30 partial sums) in
                        # the idle pproj PSUM banks instead.
                        QG = [(0, 5), (5, 10), (10, 15), (15, 20), (20, 26),
                              (26, 32)]
                        w0t = wot_pre[0]
                        yps0 = [pp.tile([128, 512], F32, tag="pproj", bufs=2,
                                        name=f"y0_{hh}") for hh in range(2)]

                        def qgroup(g, w0t=w0t, yps0=yps0):
                            for i in range(*QG[g]):
                                if i > 30:
                                    continue
                                for hh in range(2):
                                    nc.tensor.matmul(
                                        yps0[hh][:],
                                        lhsT=w0t[:, i * 128:(i + 1) * 128],
                                        rhs=ao[:, i * S + hh * 512: i * S + (hh + 1) * 512],
                                        start=(i == 0), stop=False,
                                    )

                    if h == H - 2:
                        # prefetch the first O-proj weight slabs (allocated
                        # after wq31 so the wq-slot rotation stays acyclic);
                        # their DMAs overlap the last two heads' attention
                        for ot in range(3):
                            w_pre = wp.tile([128, HD], BF16, tag="wq", bufs=3,
                                            name=f"wo{ot}")
                            nc.sync.dma_start(out=w_pre[:], in_=wo_d[ot])
                            wot_pre[ot] = w_pre

                    atile = wp.tile([128, A_TOT], BF16, tag="attn", bufs=2)

                    def score(kc, qlo, qhi, tag):
                        L = qhi - qlo
                        sp = pp.tile([128, L], F32, tag="pscore", bufs=2,
                                     name=f"sc{h}_{tag}")
                        nc.tensor.matmul(
                            sp[:], lhsT=kr_t[:, kc * 128:(kc + 1) * 128],
                            rhs=qrt[:, qlo:qhi], start=True, stop=True,
                        )
                        nc.scalar.activation(
                            atile[:, acol(kc, qlo):acol(kc, qhi)], sp[:],
                            AF.Sigmoid, scale=SCALE,
                        )

                    def av(dst, kcs, qlo, qhi):
                        # dst += sum_kc V[kc].T @ a[kc, qlo:qhi]
                        for j, kc in enumerate(kcs):
                            nc.tensor.matmul(
                                dst[:],
                                lhsT=v_all[:, kc * 128:(kc + 1) * 128],
                                rhs=atile[:, acol(kc, qlo):acol(kc, qhi)],
                                start=(j == 0), stop=(j == len(kcs) - 1),
                            )

                    def denom(w, kcs, qlo, qhi):
                        # rb = 1 / sum_k sigmoid over window (broadcast on 128 parts)
                        L = qhi - qlo
                        asum = wp.tile([128, L], BF16, tag="asum", bufs=1,
                                       name=f"as{h}_{w}")
                        nc.vector.tensor_add(
                            asum[:], atile[:, acol(kcs[0], qlo):acol(kcs[0], qhi)],
                            atile[:, acol(kcs[1], qlo):acol(kcs[1], qhi)])
                        for kc in kcs[2:]:
                            nc.vector.tensor_add(
                                asum[:], asum[:],
                                atile[:, acol(kc, qlo):acol(kc, qhi)])
                        rb = wp.tile([128, L], F32, tag="rb", bufs=2,
                                     name=f"rb{h}_{w}")
                        nc.gpsimd.partition_all_reduce(
                            rb[:], asum[:], channels=128,
                            reduce_op=bass_isa.ReduceOp.add)
                        nc.vector.reciprocal_approx_fast(rb[:], rb[:])
                        return rb

                    _ncnt = [0]

                    def ntmp(L, tag):
                        _ncnt[0] += 1
                        return wp.tile([128, L], BF16, tag=tag, bufs=2,
                                       name=f"nt{h}_{_ncnt[0]}")

                    def blend(qlo, n_new):
                        # ao[qlo:qlo+256] += alpha * (n_new - ao[qlo:qlo+256])
                        sl = aoh[:, qlo:qlo + 256]
                        _ncnt[0] += 1
                        t2 = wp.tile([128, 256], BF16, tag="bl2", bufs=1,
                                     name=f"bl2_{h}_{_ncnt[0]}")
                        nc.vector.tensor_sub(t2[:], n_new[:], sl)
                        t3 = wp.tile([128, 256], BF16, tag="bl3", bufs=1,
                                     name=f"bl3_{h}_{_ncnt[0]}")
                        nc.vector.tensor_mul(t3[:], t2[:], alpha_t[:])
                        nc.vector.tensor_add(sl, sl, t3[:])

                    aoh = ao[:, h * S:(h + 1) * S]

                    # --- window 0 ---
                    score(0, 0, 512, "a0")
                    score(1, 0, 512, "a1")
                    qgroup(0)
                    score(2, 0, 512, "a2")
                    score(3, 0, 512, "a3")
                    qgroup(1)
                    pavA = pp.tile([128, 512], F32, tag="pav", bufs=3, name=f"A{h}")
                    av(pavA, [0, 1], 0, 512)
                    pavB = pp.tile([128, 512], F32, tag="pav", bufs=3, name=f"B{h}")
                    av(pavB, [2, 3], 0, 512)
                    rb0 = denom(0, [0, 1, 2, 3], 0, 512)
                    nA = ntmp(512, "n1")
                    nc.vector.tensor_mul(nA[:], pavA[:], rb0[:])
                    nB = ntmp(512, "n2")
                    nc.vector.tensor_mul(nB[:], pavB[:], rb0[:])
                    nc.vector.tensor_add(aoh[:, 0:512], nA[:], nB[:])
                    qgroup(2)

                    # --- window 1 ---
                    score(4, 256, 768, "a4")
                    score(5, 256, 768, "a5")
                    qgroup(3)
                    score(2, 512, 768, "a2b")
                    score(3, 512, 768, "a3b")
                    pavC1 = pp.tile([128, 512], F32, tag="pav", bufs=3, name=f"C1{h}")
                    av(pavC1, [4, 5], 256, 768)
                    pavC2 = pp.tile([128, 256], F32, tag="pav", bufs=3, name=f"C2{h}")
                    av(pavC2, [2, 3], 512, 768)
                    rb1 = denom(1, [2, 3, 4, 5], 256, 768)
                    nB1 = ntmp(256, "n1")
                    nc.vector.tensor_mul(nB1[:], pavB[:, 256:512], rb1[:, 0:256])
                    nC1 = ntmp(512, "n2")
                    nc.vector.tensor_mul(nC1[:], pavC1[:], rb1[:])
                    n1a = ntmp(256, "n3")
                    nc.vector.tensor_add(n1a[:], nB1[:], nC1[:, 0:256])
                    blend(256, n1a)
                    nC2 = ntmp(256, "n1")
                    nc.vector.tensor_mul(nC2[:], pavC2[:], rb1[:, 256:512])
                    nc.vector.tensor_add(aoh[:, 512:768], nC2[:], nC1[:, 256:512])
                    qgroup(4)

                    # --- window 2 ---
                    score(6, 512, 1024, "a6")
                    score(7, 512, 1024, "a7")
                    qgroup(5)
                    score(4, 768, 1024, "a4b")
                    score(5, 768, 1024, "a5b")
                    pavD = pp.tile([128, 512], F32, tag="pav", bufs=3, name=f"D{h}")
                    av(pavD, [6, 7], 512, 1024)
                    # evacuate Q(h+1) now (qps stopped in qgroup(5)); the ACT
                    # copies and rope DVE work hide under the w2/w3 PE stream
                    if h + 1 < H:
                        qraw = wp.tile([128, S], BF16, tag="qraw", bufs=1,
                                       name=f"qraw{h+1}")
                        for hh in range(2):
                            nc.scalar.copy(qraw[:, hh * 512:(hh + 1) * 512], qps[hh][:])
                        mc_next = rope_mc(qraw)
                    rb2 = denom(2, [4, 5, 6, 7], 512, 1024)
                    nC1b = ntmp(256, "n1")
                    nc.vector.tensor_mul(nC1b[:], pavC1[:, 256:512], rb2[:, 0:256])
                    nD = ntmp(512, "n2")
                    nc.vector.tensor_mul(nD[:], pavD[:], rb2[:])
                    n2a = ntmp(256, "n3")
                    nc.vector.tensor_add(n2a[:], nC1b[:], nD[:, 0:256])
                    blend(512, n2a)
                    pavF = pp.tile([128, 256], F32, tag="pav", bufs=3, name=f"F{h}")
                    av(pavF, [4, 5], 768, 1024)
                    if h + 1 < H:
                        qrt_next = wp.tile([128, S], BF16, tag="qrt", bufs=2,
                                           name=f"qrt{h+1}")
                        rope_rot(qrt_next, qraw, mc_next, f"q{h+1}")
                        qrt_holder[0] = qrt_next
                    nF = ntmp(256, "n1")
                    nc.vector.tensor_mul(nF[:], pavF[:], rb2[:, 256:512])
                    nc.vector.tensor_add(aoh[:, 768:1024], nD[:, 256:512], nF[:])

                    # --- window 3 ---
                    rb3 = denom(3, [6, 7], 768, 1024)
                    n3 = ntmp(256, "n2")
                    nc.vector.tensor_mul(n3[:], pavD[:, 256:512], rb3[:])
                    blend(768, n3)

                    if h == H - 1:
                        # close out O-proj tile 0 with the head-31 term
                        for hh in range(2):
                            nc.tensor.matmul(
                                yps0[hh][:],
                                lhsT=wot_pre[0][:, 31 * 128:32 * 128],
                                rhs=ao[:, 31 * S + hh * 512: 31 * S + (hh + 1) * 512],
                                start=False, stop=True,
                            )
                        yo0 = wp.tile([128, S], F32, tag="yo", bufs=1,
                                      name="yo_t0")
                        nc.scalar.copy(yo0[:, 0:512], yps0[0][:])
                        nc.vector.tensor_copy(yo0[:, 512:1024], yps0[1][:])
                        nc.sync.dma_start(out=y_d[0:128, :], in_=yo0[:])

              # ---- phase 3: O-projection, transposed (yT = Wo @ aoT) ----
              # psum pool `pp` is closed; open a fresh one for y tiles.
              with tc.tile_pool(name="ops", bufs=1, space="PSUM") as opp:
                    for ot in range(1, 32):
                        if ot in wot_pre:
                            wot = wot_pre[ot]
                        else:
                            wot = wp.tile([128, HD], BF16, tag="wq", bufs=3,
                                          name=f"wo{ot}")
                            nc.sync.dma_start(out=wot[:], in_=wo_d[ot])
                        yps = [opp.tile([128, 512], F32, tag="yps", bufs=6,
                                        name=f"yp{ot}_{hh}") for hh in range(2)]
                        for i in range(NI):
                            for hh in range(2):
                                nc.tensor.matmul(
                                    yps[hh][:],
                                    lhsT=wot[:, i * 128:(i + 1) * 128],
                                    rhs=ao[:, i * S + hh * 512: i * S + (hh + 1) * 512],
                                    start=(i == 0), stop=(i == NI - 1),
                                )
                        yo = wp.tile([128, S], F32, tag="yo", bufs=1, name=f"yo{ot}")
                        nc.scalar.copy(yo[:, 0:512], yps[0][:])
                        nc.vector.tensor_copy(yo[:, 512:1024], yps[1][:])
                        nc.sync.dma_start(
                            out=y_d[ot * 128:(ot + 1) * 128, :], in_=yo[:],
                        )
    nc.compile()
    return nc


def prep_inputs(x, Wq, Wk, Wv, Wo):
    """Host-side: transpose/tile/cast so every device DMA is contiguous."""
    bf = ml_dtypes.bfloat16
    xT = np.ascontiguousarray(np.transpose(x, (0, 2, 1))).astype(bf)   # [B,4096,1024]
    # wq[h,p,i*128+c] = Wq[h*128+c, i*128+p]
    wq = np.ascontiguousarray(
        Wq.reshape(H, 128, NI, 128).transpose(0, 3, 2, 1).reshape(H, 128, HD)
    ).astype(bf)
    # wk[p, i*128+c] = Wk[c, i*128+p]
    wk = np.ascontiguousarray(
        Wk.reshape(128, NI, 128).transpose(2, 1, 0).reshape(128, HD)
    ).astype(bf)
    wv = np.ascontiguousarray(
        Wv.reshape(128, NI, 128).transpose(2, 1, 0).reshape(128, HD)
    ).astype(bf)
    # wo[ot, p, i*128+c] = Wo[ot*128+c, i*128+p]
    wo = np.ascontiguousarray(
        Wo.reshape(32, 128, NI, 128).transpose(0, 3, 2, 1).reshape(32, 128, HD)
    ).astype(bf)
    cos, sin = _rope_cache_np(S, DH)
    cosT = np.ascontiguousarray(cos.T).astype(bf)                      # [128,1024]
    sinS = np.ascontiguousarray(sin.T).astype(bf)
    rotm = np.zeros((128, 128), dtype=np.float32)
    rotm[np.arange(64) + 64, np.arange(64)] = -1.0
    rotm[np.arange(64), np.arange(64) + 64] = 1.0
    rotm = rotm.astype(bf)
    alphaB = np.tile(
        np.linspace(0.0, 1.0, 256, dtype=np.float32)[None, :], (128, 1)
    ).astype(bf)
    ident = np.eye(128, dtype=np.float32).astype(bf)
    shared = dict(wq=wq, wk=wk, wv=wv, wo=wo, cosT=cosT, sinS=sinS, alphaB=alphaB,
                  rotm=rotm, ident=ident)
    in_maps = [dict(xT=xT[b], **shared) for b in range(B)]
    return in_maps


def kernel(x, Wq, Wk, Wv, Wo):
    if "nc" not in _CACHE:
        _CACHE["nc"] = build_nc()
    nc = _CACHE["nc"]
    in_maps = prep_inputs(
        np.asarray(x, dtype=np.float32),
        np.asarray(Wq, dtype=np.float32),
        np.asarray(Wk, dtype=np.float32),
        np.asarray(Wv, dtype=np.float32),
        np.asarray(Wo, dtype=np.float32),
    )
    res = run_bass_kernel_spmd(nc, in_maps, core_ids=list(range(B)))
    out = np.stack(
        [np.ascontiguousarray(np.asarray(res.results[b]["y"]).T) for b in range(B)],
        axis=0,
    )
    return out.astype(np.float32)


if __name__ == "__main__":
    rng = np.random.default_rng(0)
    x = rng.standard_normal((B, S, HD), dtype=np.float32)
    Wq = (rng.standard_normal((HD, HD), dtype=np.float32) * 0.02)
    Wk = (rng.standard_normal((DH, HD), dtype=np.float32) * 0.02)
    Wv = (rng.standard_normal((DH, HD), dtype=np.float32) * 0.02)
    Wo = (rng.standard_normal((HD, HD), dtype=np.float32) * 0.02)
    y = kernel(x=x, Wq=Wq, Wk=Wk, Wv=Wv, Wo=Wo)
    print("out", y.shape, y.dtype, float(np.abs(y).mean()))


# revision 5
# speedup vs baseline: 1.2457x; 1.2457x over previous
"""Trainium2 Bass kernel for nn_AdvancedFastMQA — v2.

Data-parallel over batch B=8 across 8 NeuronCores. Transposed dataflow
(no on-device transposes except V). Per-core PE-cycle cuts vs v1:

 - Sliding-window overlap sharing: score tiles and attention@V partial
   sums for the k-chunk regions shared by adjacent windows are computed
   once (40 unit tiles instead of 52 for both scores and AV). Window
   outputs are assembled from 2-chunk PSUM partials (A,B,C1,C2,D,F) with
   cheap DVE combines.
 - Denominator: instead of M=1 ones-vector matmuls per k-chunk (same
   streaming cost as scores, zero useful flops), the k-chunk sigmoid
   tiles are summed on DVE and a single ones[128x128] matmul per window
   produces the partition-sum already broadcast across 128 partitions
   (also kills the gpsimd broadcast).
 - O-projection computed transposed: yT[o,t] = sum_i wo_tile[i].T @ ao_i
   with the weight stationary for 2 matmuls each, PSUM double-buffered;
   host transposes the [4096,1024] result back.
 - Q-projection of head h+1 is interleaved into attention of head h so
   the PE never waits on ACT sigmoids.

Windows (S=1024, window=512, stride 256):
  w0: k,q in [0,512); w1: k,q [256,768); w2: k,q [512,1024);
  w3: k,q [768,1024). Blend regions: [256,512) w0/w1, [512,768) w1/w2,
  [768,1024) w2/w3, alpha = linspace(0,1,256).

AV partial-sum plan (per head, PSUM tiles, kc = 128-wide k chunks):
  A  = kc0+kc1 over q[0:512)      B  = kc2+kc3 over q[0:512)
  C1 = kc4+kc5 over q[256:768)    C2 = kc2+kc3 over q[512:768)
  D  = kc6+kc7 over q[512:1024)   F  = kc4+kc5 over q[768:1024)
  u0 = A+B; u1 = B[256:512)+C1 | C2+C1[512:768); u2 = C1+D | D+F;
  u3 = D[768:1024).
"""

import sys

for _p in ("/opt/trn_rl_repo", "/opt/pypackages"):
    if _p not in sys.path:
        sys.path.append(_p)

import numpy as np
import ml_dtypes

import concourse.bacc as bacc
import concourse.tile as tile
import concourse.mybir as mybir
import concourse.bass_isa as bass_isa
from concourse.bass_utils import run_bass_kernel_spmd

BF16 = mybir.dt.bfloat16
F32 = mybir.dt.float32
AF = mybir.ActivationFunctionType

B, S, HD = 8, 1024, 4096
H, DH = 32, 128
WINDOW = 512
SCALE = 1.0 / float(np.sqrt(DH))
ROPE_BASE = 10000.0
NI = HD // 128          # 32 contraction chunks
NT = S // 128           # 8 token chunks

# a-tile (sigmoid) layout: per kc the union of q-ranges that need it.
A_QLO = [0, 0, 0, 0, 256, 256, 512, 512]
A_W = [512, 512, 768, 768, 768, 768, 512, 512]
A_OFF = [0, 512, 1024, 1792, 2560, 3328, 4096, 4608]
A_TOT = 5120

_CACHE = {}


def _rope_cache_np(S_, D_, base=ROPE_BASE):
    inv_freq = 1.0 / (base ** (np.arange(0, D_, 2, dtype=np.float32) / D_))
    t = np.arange(S_, dtype=np.float32)
    f = np.outer(t, inv_freq)
    cos = np.zeros((S_, D_), dtype=np.float32)
    sin = np.zeros((S_, D_), dtype=np.float32)
    cos[:, 0::2] = np.cos(f)
    cos[:, 1::2] = np.cos(f)
    sin[:, 0::2] = np.sin(f)
    sin[:, 1::2] = np.sin(f)
    return cos, sin


def build_nc():
    nc = bacc.Bacc("TRN2", debug=False, target_bir_lowering=False)

    xT_d = nc.dram_tensor("xT", [HD, S], BF16, kind="ExternalInput").ap()
    wq_d = nc.dram_tensor("wq", [H, 128, HD], BF16, kind="ExternalInput").ap()
    wk_d = nc.dram_tensor("wk", [128, HD], BF16, kind="ExternalInput").ap()
    wv_d = nc.dram_tensor("wv", [128, HD], BF16, kind="ExternalInput").ap()
    wo_d = nc.dram_tensor("wo", [32, 128, HD], BF16, kind="ExternalInput").ap()
    cos_d = nc.dram_tensor("cosT", [128, S], BF16, kind="ExternalInput").ap()
    sin_d = nc.dram_tensor("sinS", [128, S], BF16, kind="ExternalInput").ap()
    alpha_d = nc.dram_tensor("alphaB", [128, 256], BF16, kind="ExternalInput").ap()
    rotm_d = nc.dram_tensor("rotm", [128, 128], BF16, kind="ExternalInput").ap()
    iden_d = nc.dram_tensor("ident", [128, 128], BF16, kind="ExternalInput").ap()
    y_d = nc.dram_tensor("y", [HD, S], F32, kind="ExternalOutput").ap()

    with tile.TileContext(nc) as tc:
        with tc.tile_pool(name="consts", bufs=1) as cp:
            xt = cp.tile([128, NI * S], BF16)              # 64KB/part
            cos_t = cp.tile([128, S], BF16)
            sin_t = cp.tile([128, S], BF16)
            alpha_t = cp.tile([128, 256], BF16)
            ones_t = cp.tile([128, 128], BF16)
            nc.vector.memset(ones_t[:], 1.0)
            rotm_t = cp.tile([128, 128], BF16)
            iden_t = cp.tile([128, 128], BF16)

            kr_t = cp.tile([128, S], BF16)                 # roped K
            v_all = cp.tile([128, NT * 128], BF16)         # V as 8 lhsT tiles
            ao = cp.tile([128, H * S], BF16)               # attention out, 64KB/part

            with tc.tile_pool(name="work", bufs=1) as wp:
              with tc.tile_pool(name="ps", bufs=1, space="PSUM") as pp:

                def rope_mc(src):
                    mc = wp.tile([128, S], BF16, tag="rope_mc", bufs=1)
                    nc.vector.tensor_mul(mc[:], src[:], cos_t[:])
                    return mc

                def rope_rot(dst, src, mc, tag):
                    for rh in range(2):
                        rp = pp.tile([128, 512], F32, tag="pden", bufs=1,
                                     name=f"rot_{tag}_{rh}")
                        nc.tensor.matmul(
                            rp[:], lhsT=rotm_t[:],
                            rhs=src[:, rh * 512:(rh + 1) * 512],
                            start=True, stop=True,
                        )
                        ms = wp.tile([128, 512], BF16, tag="rope_ms", bufs=1)
                        nc.vector.tensor_mul(ms[:], rp[:], sin_t[:, rh * 512:(rh + 1) * 512])
                        nc.vector.tensor_add(
                            dst[:, rh * 512:(rh + 1) * 512],
                            mc[:, rh * 512:(rh + 1) * 512], ms[:],
                        )

                def rope(dst, src, tag):
                    # dst = src*cos + rotate_half(src)*sin; rotate via PE
                    rope_rot(dst, src, rope_mc(src), tag)

                # ---- phase 1: interleaved K / VT / Q-head0 projections ----
                # DMA order matters: the first matmuls need the weights and
                # x chunk 0, so those go first; bulk x and rope consts after.
                wk_t = wp.tile([128, HD], BF16, tag="wq", bufs=3, name="wk")
                wv_t = wp.tile([128, HD], BF16, tag="wq", bufs=3, name="wv")
                wq0_t = wp.tile([128, HD], BF16, tag="wq", bufs=3, name="wq0")
                def wpiece(p):
                    c0 = p * 1024
                    nc.sync.dma_start(out=wk_t[:, c0:c0 + 1024],
                                      in_=wk_d[:, c0:c0 + 1024])
                    nc.sync.dma_start(out=wv_t[:, c0:c0 + 1024],
                                      in_=wv_d[:, c0:c0 + 1024])
                    nc.sync.dma_start(out=wq0_t[:, c0:c0 + 1024],
                                      in_=wq_d[0, :, c0:c0 + 1024])

                # DMA packets drain the queue ~in order, so emit transfers in
                # exact consumption order: weight piece p just before the x
                # chunks that use it.
                wpiece(0)
                for i in range(NI):
                    nc.sync.dma_start(
                        out=xt[:, i * S:(i + 1) * S], in_=xT_d[i * 128:(i + 1) * 128, :]
                    )
                    if i == 5:
                        wpiece(1)
                    if i == 13:
                        wpiece(2)
                    if i == 21:
                        wpiece(3)
                    if i == 8:
                        nc.sync.dma_start(out=cos_t[:], in_=cos_d[:])
                        nc.sync.dma_start(out=sin_t[:], in_=sin_d[:])
                    if i == 10:
                        nc.sync.dma_start(out=alpha_t[:], in_=alpha_d[:])
                        nc.sync.dma_start(out=rotm_t[:], in_=rotm_d[:])
                        nc.sync.dma_start(out=iden_t[:], in_=iden_d[:])
                kps = [pp.tile([128, 512], F32, tag="pscore", bufs=2, name=f"kp{hh}") for hh in range(2)]
                vps = [pp.tile([128, 512], F32, tag="pav", bufs=3, name=f"vp{hh}") for hh in range(2)]
                qps0 = [pp.tile([128, 512], F32, tag="pproj", bufs=2, name=f"qp0{hh}") for hh in range(2)]
                for i in range(NI):
                    st_ = (i == 0)
                    sp_ = (i == NI - 1)
                    for hh in range(2):
                        rhs = xt[:, i * S + hh * 512: i * S + (hh + 1) * 512]
                        nc.tensor.matmul(kps[hh][:], lhsT=wk_t[:, i * 128:(i + 1) * 128],
                                         rhs=rhs, start=st_, stop=sp_)
                        nc.tensor.matmul(vps[hh][:], lhsT=wv_t[:, i * 128:(i + 1) * 128],
                                         rhs=rhs, start=st_, stop=sp_)
                        nc.tensor.matmul(qps0[hh][:], lhsT=wq0_t[:, i * 128:(i + 1) * 128],
                                         rhs=rhs, start=st_, stop=sp_)
                kraw = wp.tile([128, S], BF16, tag="kraw", bufs=1)
                vtraw = wp.tile([128, S], BF16, tag="qrt", bufs=2)
                qraw0 = wp.tile([128, S], BF16, tag="qraw", bufs=1)
                for hh in range(2):
                    nc.scalar.copy(kraw[:, hh * 512:(hh + 1) * 512], kps[hh][:])
                    nc.scalar.copy(vtraw[:, hh * 512:(hh + 1) * 512], vps[hh][:])
                    nc.scalar.copy(qraw0[:, hh * 512:(hh + 1) * 512], qps0[hh][:])
                rope(kr_t, kraw, "k")
                for t in range(NT):
                    tp = pp.tile([128, 128], BF16, tag="pscore", bufs=2, name=f"vtp{t}")
                    nc.tensor.transpose(tp[:], vtraw[:, t * 128:(t + 1) * 128], iden_t[:])
                    nc.scalar.copy(v_all[:, t * 128:(t + 1) * 128], tp[:])
                qrt0 = wp.tile([128, S], BF16, tag="qrt", bufs=2)
                rope(qrt0, qraw0, "q0")

                # ---- phase 2: per-head attention + interleaved Q proj(h+1) ----
                def acol(kc, q):
                    return A_OFF[kc] + q - A_QLO[kc]

                qrt_holder = [qrt0]
                wot_pre = {}
                yps0 = None
                for h in range(H):
                    qrt = qrt_holder[0]

                    # Q projection emitters for head h+1, in 8 groups of 4 i's
                    if h + 1 < H:
                        wq_t = wp.tile([128, HD], BF16, tag="wq", bufs=3)
                        nc.sync.dma_start(out=wq_t[:], in_=wq_d[h + 1])
                        qps = [pp.tile([128, 512], F32, tag="pproj", bufs=2,
                                       name=f"qp{h+1}_{hh}") for hh in range(2)]

                        QG = [(0, 5), (5, 10), (10, 15), (15, 20), (20, 26),
                              (26, 32)]

                        def qgroup(g, wq_t=wq_t, qps=qps):
                            for i in range(*QG[g]):
                                for hh in range(2):
                                    nc.tensor.matmul(
                                        qps[hh][:],
                                        lhsT=wq_t[:, i * 128:(i + 1) * 128],
                                        rhs=xt[:, i * S + hh * 512: i * S + (hh + 1) * 512],
                                        start=(i == 0), stop=(i == NI - 1),
                                    )
                    else:
                        # last head: no Q projection to interleave — fill the
                        # PE with O-proj tile 0 (heads 0..30 partial sums) in
                        # the idle pproj PSUM banks instead.
                        QG = [(0, 5), (5, 10), (10, 15), (15, 20), (20, 26),
                              (26, 32)]
                        w0t = wot_pre[0]
                        yps0 = [pp.tile([128, 512], F32, tag="pproj", bufs=2,
                                        name=f"y0_{hh}") for hh in range(2)]

                        def qgroup(g, w0t=w0t, yps0=yps0):
                            for i in range(*QG[g]):
                                if i > 30:
                                    continue
                                for hh in range(2):
                                    nc.tensor.matmul(
                                        yps0[hh][:],
                                        lhsT=w0t[:, i * 128:(i + 1) * 128],
                                        rhs=ao[:, i * S + hh * 512: i * S + (hh + 1) * 512],
                                        start=(i == 0), stop=False,
                                    )

                    if h == H - 2:
                        # prefetch the first O-proj weight slabs (allocated
                        # after wq31 so the wq-slot rotation stays acyclic);
                        # their DMAs overlap the last two heads' attention
                        for ot in range(3):
                            w_pre = wp.tile([128, HD], BF16, tag="wq", bufs=3,
                                            name=f"wo{ot}")
                            nc.sync.dma_start(out=w_pre[:], in_=wo_d[ot])
                            wot_pre[ot] = w_pre

                    atile = wp.tile([128, A_TOT], BF16, tag="attn", bufs=2)

                    def score(kc, qlo, qhi, tag):
                        L = qhi - qlo
                        sp = pp.tile([128, L], F32, tag="pscore", bufs=2,
                                     name=f"sc{h}_{tag}")
                        nc.tensor.matmul(
                            sp[:], lhsT=kr_t[:, kc * 128:(kc + 1) * 128],
                            rhs=qrt[:, qlo:qhi], start=True, stop=True,
                        )
                        nc.scalar.activation(
                            atile[:, acol(kc, qlo):acol(kc, qhi)], sp[:],
                            AF.Sigmoid, scale=SCALE,
                        )

                    def av(dst, kcs, qlo, qhi):
                        # dst += sum_kc V[kc].T @ a[kc, qlo:qhi]
                        for j, kc in enumerate(kcs):
                            nc.tensor.matmul(
                                dst[:],
                                lhsT=v_all[:, kc * 128:(kc + 1) * 128],
                                rhs=atile[:, acol(kc, qlo):acol(kc, qhi)],
                                start=(j == 0), stop=(j == len(kcs) - 1),
                            )

                    def denom(w, kcs, qlo, qhi):
                        # rb = 1 / sum_k sigmoid over window (broadcast on 128 parts)
                        L = qhi - qlo
                        asum = wp.tile([128, L], BF16, tag="asum", bufs=1,
                                       name=f"as{h}_{w}")
                        nc.vector.tensor_add(
                            asum[:], atile[:, acol(kcs[0], qlo):acol(kcs[0], qhi)],
                            atile[:, acol(kcs[1], qlo):acol(kcs[1], qhi)])
                        for kc in kcs[2:]:
                            nc.vector.tensor_add(
                                asum[:], asum[:],
                                atile[:, acol(kc, qlo):acol(kc, qhi)])
                        dn = pp.tile([128, L], F32, tag="pden", bufs=1,
                                     name=f"dn{h}_{w}")
                        nc.tensor.matmul(dn[:], lhsT=ones_t[:], rhs=asum[:],
                                         start=True, stop=True)
                        rb = wp.tile([128, L], F32, tag="rb", bufs=2,
                                     name=f"rb{h}_{w}")
                        nc.vector.reciprocal_approx_fast(rb[:], dn[:])
                        return rb

                    _ncnt = [0]

                    def ntmp(L, tag):
                        _ncnt[0] += 1
                        return wp.tile([128, L], BF16, tag=tag, bufs=2,
                                       name=f"nt{h}_{_ncnt[0]}")

                    def blend(qlo, n_new):
                        # ao[qlo:qlo+256] += alpha * (n_new - ao[qlo:qlo+256])
                        sl = aoh[:, qlo:qlo + 256]
                        _ncnt[0] += 1
                        t2 = wp.tile([128, 256], BF16, tag="bl2", bufs=1,
                                     name=f"bl2_{h}_{_ncnt[0]}")
                        nc.vector.tensor_sub(t2[:], n_new[:], sl)
                        t3 = wp.tile([128, 256], BF16, tag="bl3", bufs=1,
                                     name=f"bl3_{h}_{_ncnt[0]}")
                        nc.vector.tensor_mul(t3[:], t2[:], alpha_t[:])
                        nc.vector.tensor_add(sl, sl, t3[:])

                    aoh = ao[:, h * S:(h + 1) * S]

                    # --- window 0 ---
                    score(0, 0, 512, "a0")
                    score(1, 0, 512, "a1")
                    qgroup(0)
                    score(2, 0, 512, "a2")
                    score(3, 0, 512, "a3")
                    qgroup(1)
                    pavA = pp.tile([128, 512], F32, tag="pav", bufs=3, name=f"A{h}")
                    av(pavA, [0, 1], 0, 512)
                    pavB = pp.tile([128, 512], F32, tag="pav", bufs=3, name=f"B{h}")
                    av(pavB, [2, 3], 0, 512)
                    rb0 = denom(0, [0, 1, 2, 3], 0, 512)
                    nA = ntmp(512, "n1")
                    nc.vector.tensor_mul(nA[:], pavA[:], rb0[:])
                    nB = ntmp(512, "n2")
                    nc.vector.tensor_mul(nB[:], pavB[:], rb0[:])
                    nc.vector.tensor_add(aoh[:, 0:512], nA[:], nB[:])
                    qgroup(2)

                    # --- window 1 ---
                    score(4, 256, 768, "a4")
                    score(5, 256, 768, "a5")
                    qgroup(3)
                    score(2, 512, 768, "a2b")
                    score(3, 512, 768, "a3b")
                    pavC1 = pp.tile([128, 512], F32, tag="pav", bufs=3, name=f"C1{h}")
                    av(pavC1, [4, 5], 256, 768)
                    pavC2 = pp.tile([128, 256], F32, tag="pav", bufs=3, name=f"C2{h}")
                    av(pavC2, [2, 3], 512, 768)
                    rb1 = denom(1, [2, 3, 4, 5], 256, 768)
                    nB1 = ntmp(256, "n1")
                    nc.vector.tensor_mul(nB1[:], pavB[:, 256:512], rb1[:, 0:256])
                    nC1 = ntmp(512, "n2")
                    nc.vector.tensor_mul(nC1[:], pavC1[:], rb1[:])
                    n1a = ntmp(256, "n3")
                    nc.vector.tensor_add(n1a[:], nB1[:], nC1[:, 0:256])
                    blend(256, n1a)
                    nC2 = ntmp(256, "n1")
                    nc.vector.tensor_mul(nC2[:], pavC2[:], rb1[:, 256:512])
                    nc.vector.tensor_add(aoh[:, 512:768], nC2[:], nC1[:, 256:512])
                    qgroup(4)

                    # --- window 2 ---
                    score(6, 512, 1024, "a6")
                    score(7, 512, 1024, "a7")
                    qgroup(5)
                    score(4, 768, 1024, "a4b")
                    score(5, 768, 1024, "a5b")
                    pavD = pp.tile([128, 512], F32, tag="pav", bufs=3, name=f"D{h}")
                    av(pavD, [6, 7], 512, 1024)
                    # evacuate Q(h+1) now (qps stopped in qgroup(5)); the ACT
                    # copies and rope DVE work hide under the w2/w3 PE stream
                    if h + 1 < H:
                        qraw = wp.tile([128, S], BF16, tag="qraw", bufs=1,
                                       name=f"qraw{h+1}")
                        for hh in range(2):
                            nc.scalar.copy(qraw[:, hh * 512:(hh + 1) * 512], qps[hh][:])
                        mc_next = rope_mc(qraw)
                    rb2 = denom(2, [4, 5, 6, 7], 512, 1024)
                    nC1b = ntmp(256, "n1")
                    nc.vector.tensor_mul(nC1b[:], pavC1[:, 256:512], rb2[:, 0:256])
                    nD = ntmp(512, "n2")
                    nc.vector.tensor_mul(nD[:], pavD[:], rb2[:])
                    n2a = ntmp(256, "n3")
                    nc.vector.tensor_add(n2a[:], nC1b[:], nD[:, 0:256])
                    blend(512, n2a)
                    pavF = pp.tile([128, 256], F32, tag="pav", bufs=3, name=f"F{h}")
                    av(pavF, [4, 5], 768, 1024)
                    if h + 1 < H:
                        qrt_next = wp.tile([128, S], BF16, tag="qrt", bufs=2,
                                           name=f"qrt{h+1}")
                        rope_rot(qrt_next, qraw, mc_next, f"q{h+1}")
                        qrt_holder[0] = qrt_next
                    nF = ntmp(256, "n1")
                    nc.vector.tensor_mul(nF[:], pavF[:], rb2[:, 256:512])
                    nc.vector.tensor_add(aoh[:, 768:1024], nD[:, 256:512], nF[:])

                    # --- window 3 ---
                    rb3 = denom(3, [6, 7], 768, 1024)
                    n3 = ntmp(256, "n2")
                    nc.vector.tensor_mul(n3[:], pavD[:, 256:512], rb3[:])
                    blend(768, n3)

                    if h == H - 1:
                        # close out O-proj tile 0 with the head-31 term
                        for hh in range(2):
                            nc.tensor.matmul(
                                yps0[hh][:],
                                lhsT=wot_pre[0][:, 31 * 128:32 * 128],
                                rhs=ao[:, 31 * S + hh * 512: 31 * S + (hh + 1) * 512],
                                start=False, stop=True,
                            )
                        yo0 = wp.tile([128, S], F32, tag="yo", bufs=1,
                                      name="yo_t0")
                        nc.scalar.copy(yo0[:, 0:512], yps0[0][:])
                        nc.vector.tensor_copy(yo0[:, 512:1024], yps0[1][:])
                        nc.sync.dma_start(out=y_d[0:128, :], in_=yo0[:])

              # ---- phase 3: O-projection, transposed (yT = Wo @ aoT) ----
              # psum pool `pp` is closed; open a fresh one for y tiles.
              with tc.tile_pool(name="ops", bufs=1, space="PSUM") as opp:
                    for ot in range(1, 32):
                        if ot in wot_pre:
                            wot = wot_pre[ot]
                        else:
                            wot = wp.tile([128, HD], BF16, tag="wq", bufs=3,
                                          name=f"wo{ot}")
                            nc.sync.dma_start(out=wot[:], in_=wo_d[ot])
                        yps = [opp.tile([128, 512], F32, tag="yps", bufs=6,
                                        name=f"yp{ot}_{hh}") for hh in range(2)]
                        for i in range(NI):
                            for hh in range(2):
                                nc.tensor.matmul(
                                    yps[hh][:],
                                    lhsT=wot[:, i * 128:(i + 1) * 128],
                                    rhs=ao[:, i * S + hh * 512: i * S + (hh + 1) * 512],
                                    start=(i == 0), stop=(i == NI - 1),
                                )
                        yo = wp.tile([128, S], F32, tag="yo", bufs=1, name=f"yo{ot}")
                        nc.scalar.copy(yo[:, 0:512], yps[0][:])
                        nc.vector.tensor_copy(yo[:, 512:1024], yps[1][:])
                        nc.sync.dma_start(
                            out=y_d[ot * 128:(ot + 1) * 128, :], in_=yo[:],
                        )
    nc.compile()
    return nc


def prep_inputs(x, Wq, Wk, Wv, Wo):
    """Host-side: transpose/tile/cast so every device DMA is contiguous."""
    bf = ml_dtypes.bfloat16
    xT = np.ascontiguousarray(np.transpose(x, (0, 2, 1))).astype(bf)   # [B,4096,1024]
    # wq[h,p,i*128+c] = Wq[h*128+c, i*128+p]
    wq = np.ascontiguousarray(
        Wq.reshape(H, 128, NI, 128).transpose(0, 3, 2, 1).reshape(H, 128, HD)
    ).astype(bf)
    # wk[p, i*128+c] = Wk[c, i*128+p]
    wk = np.ascontiguousarray(
        Wk.reshape(128, NI, 128).transpose(2, 1, 0).reshape(128, HD)
    ).astype(bf)
    wv = np.ascontiguousarray(
        Wv.reshape(128, NI, 128).transpose(2, 1, 0).reshape(128, HD)
    ).astype(bf)
    # wo[ot, p, i*128+c] = Wo[ot*128+c, i*128+p]
    wo = np.ascontiguousarray(
        Wo.reshape(32, 128, NI, 128).transpose(0, 3, 2, 1).reshape(32, 128, HD)
    ).astype(bf)
    cos, sin = _rope_cache_np(S, DH)
    cosT = np.ascontiguousarray(cos.T).astype(bf)                      # [128,1024]
    sinS = np.ascontiguousarray(sin.T).astype(bf)
    rotm = np.zeros((128, 128), dtype=np.float32)
    rotm[np.arange(64) + 64, np.arange(64)] = -1.0
    rotm[np.arange(64), np.arange(64) + 64] = 1.0
    rotm = rotm.astype(bf)
    alphaB = np.tile(
        np.linspace(0.0, 1.0, 256, dtype=np.float32)[None, :], (128, 1)
    ).astype(bf)
    ident = np.eye(128, dtype=np.float32).astype(bf)
    shared = dict(wq=wq, wk=wk, wv=wv, wo=wo, cosT=cosT, sinS=sinS, alphaB=alphaB,
                  rotm=rotm, ident=ident)
    in_maps = [dict(xT=xT[b], **shared) for b in range(B)]
    return in_maps


def kernel(x, Wq, Wk, Wv, Wo):
    if "nc" not in _CACHE:
        _CACHE["nc"] = build_nc()
    nc = _CACHE["nc"]
    in_maps = prep_inputs(
        np.asarray(x, dtype=np.float32),
        np.asarray(Wq, dtype=np.float32),
        np.asarray(Wk, dtype=np.float32),
        np.asarray(Wv, dtype=np.float32),
        np.asarray(Wo, dtype=np.float32),
    )
    res = run_bass_kernel_spmd(nc, in_maps, core_ids=list(range(B)))
    out = np.stack(
        [np.ascontiguousarray(np.asarray(res.results[b]["y"]).T) for b in range(B)],
        axis=0,
    )
    return out.astype(np.float32)


if __name__ == "__main__":
    rng = np.random.default_rng(0)
    x = rng.standard_normal((B, S, HD), dtype=np.float32)
    Wq = (rng.standard_normal((HD, HD), dtype=np.float32) * 0.02)
    Wk = (rng.standard_normal((DH, HD), dtype=np.float32) * 0.02)
    Wv = (rng.standard_normal((DH, HD), dtype=np.float32) * 0.02)
    Wo = (rng.standard_normal((HD, HD), dtype=np.float32) * 0.02)
    y = kernel(x=x, Wq=Wq, Wk=Wk, Wv=Wv, Wo=Wo)
    print("out", y.shape, y.dtype, float(np.abs(y).mean()))


# revision 6
# speedup vs baseline: 1.2482x; 1.0020x over previous
"""Trainium2 Bass kernel for nn_AdvancedFastMQA — v2.

Data-parallel over batch B=8 across 8 NeuronCores. Transposed dataflow
(no on-device transposes except V). Per-core PE-cycle cuts vs v1:

 - Sliding-window overlap sharing: score tiles and attention@V partial
   sums for the k-chunk regions shared by adjacent windows are computed
   once (40 unit tiles instead of 52 for both scores and AV). Window
   outputs are assembled from 2-chunk PSUM partials (A,B,C1,C2,D,F) with
   cheap DVE combines.
 - Denominator: instead of M=1 ones-vector matmuls per k-chunk (same
   streaming cost as scores, zero useful flops), the k-chunk sigmoid
   tiles are summed on DVE and a single ones[128x128] matmul per window
   produces the partition-sum already broadcast across 128 partitions
   (also kills the gpsimd broadcast).
 - O-projection computed transposed: yT[o,t] = sum_i wo_tile[i].T @ ao_i
   with the weight stationary for 2 matmuls each, PSUM double-buffered;
   host transposes the [4096,1024] result back.
 - Q-projection of head h+1 is interleaved into attention of head h so
   the PE never waits on ACT sigmoids.

Windows (S=1024, window=512, stride 256):
  w0: k,q in [0,512); w1: k,q [256,768); w2: k,q [512,1024);
  w3: k,q [768,1024). Blend regions: [256,512) w0/w1, [512,768) w1/w2,
  [768,1024) w2/w3, alpha = linspace(0,1,256).

AV partial-sum plan (per head, PSUM tiles, kc = 128-wide k chunks):
  A  = kc0+kc1 over q[0:512)      B  = kc2+kc3 over q[0:512)
  C1 = kc4+kc5 over q[256:768)    C2 = kc2+kc3 over q[512:768)
  D  = kc6+kc7 over q[512:1024)   F  = kc4+kc5 over q[768:1024)
  u0 = A+B; u1 = B[256:512)+C1 | C2+C1[512:768); u2 = C1+D | D+F;
  u3 = D[768:1024).
"""

import sys

for _p in ("/opt/trn_rl_repo", "/opt/pypackages"):
    if _p not in sys.path:
        sys.path.append(_p)

import numpy as np
import ml_dtypes

import concourse.bacc as bacc
import concourse.tile as tile
import concourse.mybir as mybir
import concourse.bass_isa as bass_isa
from concourse.bass_utils import run_bass_kernel_spmd

BF16 = mybir.dt.bfloat16
F32 = mybir.dt.float32
AF = mybir.ActivationFunctionType

B, S, HD = 8, 1024, 4096
H, DH = 32, 128
WINDOW = 512
SCALE = 1.0 / float(np.sqrt(DH))
ROPE_BASE = 10000.0
NI = HD // 128          # 32 contraction chunks
NT = S // 128           # 8 token chunks

# a-tile (sigmoid) layout: per kc the union of q-ranges that need it.
A_QLO = [0, 0, 0, 0, 256, 256, 512, 512]
A_W = [512, 512, 768, 768, 768, 768, 512, 512]
A_OFF = [0, 512, 1024, 1792, 2560, 3328, 4096, 4608]
A_TOT = 5120

_CACHE = {}


def _rope_cache_np(S_, D_, base=ROPE_BASE):
    inv_freq = 1.0 / (base ** (np.arange(0, D_, 2, dtype=np.float32) / D_))
    t = np.arange(S_, dtype=np.float32)
    f = np.outer(t, inv_freq)
    cos = np.zeros((S_, D_), dtype=np.float32)
    sin = np.zeros((S_, D_), dtype=np.float32)
    cos[:, 0::2] = np.cos(f)
    cos[:, 1::2] = np.cos(f)
    sin[:, 0::2] = np.sin(f)
    sin[:, 1::2] = np.sin(f)
    return cos, sin


def build_nc():
    nc = bacc.Bacc("TRN2", debug=False, target_bir_lowering=False)

    xT_d = nc.dram_tensor("xT", [HD, S], BF16, kind="ExternalInput").ap()
    wq_d = nc.dram_tensor("wq", [H, 128, HD], BF16, kind="ExternalInput").ap()
    wk_d = nc.dram_tensor("wk", [128, HD], BF16, kind="ExternalInput").ap()
    wv_d = nc.dram_tensor("wv", [128, HD], BF16, kind="ExternalInput").ap()
    wo_d = nc.dram_tensor("wo", [32, 128, HD], BF16, kind="ExternalInput").ap()
    cos_d = nc.dram_tensor("cosT", [128, S], BF16, kind="ExternalInput").ap()
    sin_d = nc.dram_tensor("sinS", [128, S], BF16, kind="ExternalInput").ap()
    alpha_d = nc.dram_tensor("alphaB", [128, 256], BF16, kind="ExternalInput").ap()
    rotm_d = nc.dram_tensor("rotm", [128, 128], BF16, kind="ExternalInput").ap()
    iden_d = nc.dram_tensor("ident", [128, 128], BF16, kind="ExternalInput").ap()
    y_d = nc.dram_tensor("y", [HD, S], F32, kind="ExternalOutput").ap()

    with tile.TileContext(nc) as tc:
        with tc.tile_pool(name="consts", bufs=1) as cp:
            xt = cp.tile([128, NI * S], BF16)              # 64KB/part
            cos_t = cp.tile([128, S], BF16)
            sin_t = cp.tile([128, S], BF16)
            alpha_t = cp.tile([128, 256], BF16)
            ones_t = cp.tile([128, 128], BF16)
            nc.vector.memset(ones_t[:], 1.0)
            rotm_t = cp.tile([128, 128], BF16)
            iden_t = cp.tile([128, 128], BF16)

            kr_t = cp.tile([128, S], BF16)                 # roped K
            v_all = cp.tile([128, NT * 128], BF16)         # V as 8 lhsT tiles
            ao = cp.tile([128, H * S], BF16)               # attention out, 64KB/part

            with tc.tile_pool(name="work", bufs=1) as wp:
              with tc.tile_pool(name="ps", bufs=1, space="PSUM") as pp:

                def rope_mc(src):
                    mc = wp.tile([128, S], BF16, tag="rope_mc", bufs=1)
                    nc.vector.tensor_mul(mc[:], src[:], cos_t[:])
                    return mc

                def rope_rot(dst, src, mc, tag):
                    for rh in range(2):
                        rp = pp.tile([128, 512], F32, tag="pden", bufs=1,
                                     name=f"rot_{tag}_{rh}")
                        nc.tensor.matmul(
                            rp[:], lhsT=rotm_t[:],
                            rhs=src[:, rh * 512:(rh + 1) * 512],
                            start=True, stop=True,
                        )
                        ms = wp.tile([128, 512], BF16, tag="rope_ms", bufs=1)
                        nc.vector.tensor_mul(ms[:], rp[:], sin_t[:, rh * 512:(rh + 1) * 512])
                        nc.vector.tensor_add(
                            dst[:, rh * 512:(rh + 1) * 512],
                            mc[:, rh * 512:(rh + 1) * 512], ms[:],
                        )

                def rope(dst, src, tag):
                    # dst = src*cos + rotate_half(src)*sin; rotate via PE
                    rope_rot(dst, src, rope_mc(src), tag)

                # ---- phase 1: interleaved K / VT / Q-head0 projections ----
                # DMA order matters: the first matmuls need the weights and
                # x chunk 0, so those go first; bulk x and rope consts after.
                wk_t = wp.tile([128, HD], BF16, tag="wq", bufs=3, name="wk")
                wv_t = wp.tile([128, HD], BF16, tag="wq", bufs=3, name="wv")
                wq0_t = wp.tile([128, HD], BF16, tag="wq", bufs=3, name="wq0")
                def wpiece(p):
                    c0 = p * 1024
                    nc.sync.dma_start(out=wk_t[:, c0:c0 + 1024],
                                      in_=wk_d[:, c0:c0 + 1024])
                    nc.sync.dma_start(out=wv_t[:, c0:c0 + 1024],
                                      in_=wv_d[:, c0:c0 + 1024])
                    nc.sync.dma_start(out=wq0_t[:, c0:c0 + 1024],
                                      in_=wq_d[0, :, c0:c0 + 1024])

                # DMA packets drain the queue ~in order, so emit transfers in
                # exact consumption order: weight piece p just before the x
                # chunks that use it.
                for nm, wt_, wd_ in (("wk", None, None),):
                    pass
                nc.sync.dma_start(out=wk_t[:, 0:256], in_=wk_d[:, 0:256])
                nc.sync.dma_start(out=wv_t[:, 0:256], in_=wv_d[:, 0:256])
                nc.sync.dma_start(out=wq0_t[:, 0:256], in_=wq_d[0, :, 0:256])
                for i in range(NI):
                    if i == 1:
                        nc.sync.dma_start(out=wk_t[:, 256:1024],
                                          in_=wk_d[:, 256:1024])
                        nc.sync.dma_start(out=wv_t[:, 256:1024],
                                          in_=wv_d[:, 256:1024])
                        nc.sync.dma_start(out=wq0_t[:, 256:1024],
                                          in_=wq_d[0, :, 256:1024])
                    nc.sync.dma_start(
                        out=xt[:, i * S:(i + 1) * S], in_=xT_d[i * 128:(i + 1) * 128, :]
                    )
                    if i == 5:
                        wpiece(1)
                    if i == 13:
                        wpiece(2)
                    if i == 21:
                        wpiece(3)
                    if i == 8:
                        nc.sync.dma_start(out=cos_t[:], in_=cos_d[:])
                        nc.sync.dma_start(out=sin_t[:], in_=sin_d[:])
                    if i == 10:
                        nc.sync.dma_start(out=alpha_t[:], in_=alpha_d[:])
                        nc.sync.dma_start(out=rotm_t[:], in_=rotm_d[:])
                        nc.sync.dma_start(out=iden_t[:], in_=iden_d[:])
                kps = [pp.tile([128, 512], F32, tag="pscore", bufs=2, name=f"kp{hh}") for hh in range(2)]
                vps = [pp.tile([128, 512], F32, tag="pav", bufs=3, name=f"vp{hh}") for hh in range(2)]
                qps0 = [pp.tile([128, 512], F32, tag="pproj", bufs=2, name=f"qp0{hh}") for hh in range(2)]
                for i in range(NI):
                    st_ = (i == 0)
                    sp_ = (i == NI - 1)
                    for hh in range(2):
                        rhs = xt[:, i * S + hh * 512: i * S + (hh + 1) * 512]
                        nc.tensor.matmul(kps[hh][:], lhsT=wk_t[:, i * 128:(i + 1) * 128],
                                         rhs=rhs, start=st_, stop=sp_)
                        nc.tensor.matmul(vps[hh][:], lhsT=wv_t[:, i * 128:(i + 1) * 128],
                                         rhs=rhs, start=st_, stop=sp_)
                        nc.tensor.matmul(qps0[hh][:], lhsT=wq0_t[:, i * 128:(i + 1) * 128],
                                         rhs=rhs, start=st_, stop=sp_)
                kraw = wp.tile([128, S], BF16, tag="yoA", bufs=2, name="kraw")
                vtraw = wp.tile([128, S], BF16, tag="qrt", bufs=2)
                qraw0 = wp.tile([128, S], BF16, tag="qraw", bufs=1)
                for hh in range(2):
                    nc.scalar.copy(kraw[:, hh * 512:(hh + 1) * 512], kps[hh][:])
                    nc.scalar.copy(qraw0[:, hh * 512:(hh + 1) * 512], qps0[hh][:])
                    nc.vector.tensor_copy(vtraw[:, hh * 512:(hh + 1) * 512], vps[hh][:])
                rope(kr_t, kraw, "k")
                for t in range(NT):
                    tp = pp.tile([128, 128], BF16, tag="pscore", bufs=2, name=f"vtp{t}")
                    nc.tensor.transpose(tp[:], vtraw[:, t * 128:(t + 1) * 128], iden_t[:])
                    nc.scalar.copy(v_all[:, t * 128:(t + 1) * 128], tp[:])
                qrt0 = wp.tile([128, S], BF16, tag="qrt", bufs=2)
                rope(qrt0, qraw0, "q0")

                # ---- phase 2: per-head attention + interleaved Q proj(h+1) ----
                def acol(kc, q):
                    return A_OFF[kc] + q - A_QLO[kc]

                qrt_holder = [qrt0]
                wot_pre = {}
                yps0 = None
                for h in range(H):
                    qrt = qrt_holder[0]

                    # Q projection emitters for head h+1, in 8 groups of 4 i's
                    if h + 1 < H:
                        wq_t = wp.tile([128, HD], BF16, tag="wq", bufs=3)
                        nc.sync.dma_start(out=wq_t[:], in_=wq_d[h + 1])
                        qps = [pp.tile([128, 512], F32, tag="pproj", bufs=2,
                                       name=f"qp{h+1}_{hh}") for hh in range(2)]

                        QG = [(0, 5), (5, 10), (10, 15), (15, 20), (20, 26),
                              (26, 32)]

                        def qgroup(g, wq_t=wq_t, qps=qps):
                            for i in range(*QG[g]):
                                for hh in range(2):
                                    nc.tensor.matmul(
                                        qps[hh][:],
                                        lhsT=wq_t[:, i * 128:(i + 1) * 128],
                                        rhs=xt[:, i * S + hh * 512: i * S + (hh + 1) * 512],
                                        start=(i == 0), stop=(i == NI - 1),
                                    )
                    else:
                        # last head: no Q projection to interleave — fill the
                        # PE with O-proj tile 0 (heads 0..30 partial sums) in
                        # the idle pproj PSUM banks instead.
                        QG = [(0, 5), (5, 10), (10, 15), (15, 20), (20, 26),
                              (26, 32)]
                        w0t = wot_pre[0]
                        yps0 = [pp.tile([128, 512], F32, tag="pproj", bufs=2,
                                        name=f"y0_{hh}") for hh in range(2)]

                        def qgroup(g, w0t=w0t, yps0=yps0):
                            for i in range(*QG[g]):
                                if i > 30:
                                    continue
                                for hh in range(2):
                                    nc.tensor.matmul(
                                        yps0[hh][:],
                                        lhsT=w0t[:, i * 128:(i + 1) * 128],
                                        rhs=ao[:, i * S + hh * 512: i * S + (hh + 1) * 512],
                                        start=(i == 0), stop=False,
                                    )

                    if h == H - 2:
                        # prefetch the first O-proj weight slabs (allocated
                        # after wq31 so the wq-slot rotation stays acyclic);
                        # their DMAs overlap the last two heads' attention
                        for ot in range(3):
                            w_pre = wp.tile([128, HD], BF16, tag="wq", bufs=3,
                                            name=f"wo{ot}")
                            nc.sync.dma_start(out=w_pre[:], in_=wo_d[ot])
                            wot_pre[ot] = w_pre

                    atile = wp.tile([128, A_TOT], BF16, tag="attn", bufs=2)

                    def score(kc, qlo, qhi, tag):
                        L = qhi - qlo
                        sp = pp.tile([128, L], F32, tag="pscore", bufs=2,
                                     name=f"sc{h}_{tag}")
                        nc.tensor.matmul(
                            sp[:], lhsT=kr_t[:, kc * 128:(kc + 1) * 128],
                            rhs=qrt[:, qlo:qhi], start=True, stop=True,
                        )
                        nc.scalar.activation(
                            atile[:, acol(kc, qlo):acol(kc, qhi)], sp[:],
                            AF.Sigmoid, scale=SCALE,
                        )

                    def av(dst, kcs, qlo, qhi):
                        # dst += sum_kc V[kc].T @ a[kc, qlo:qhi]
                        for j, kc in enumerate(kcs):
                            nc.tensor.matmul(
                                dst[:],
                                lhsT=v_all[:, kc * 128:(kc + 1) * 128],
                                rhs=atile[:, acol(kc, qlo):acol(kc, qhi)],
                                start=(j == 0), stop=(j == len(kcs) - 1),
                            )

                    def denom(w, kcs, qlo, qhi):
                        # rb = 1 / sum_k sigmoid over window (broadcast on 128 parts)
                        L = qhi - qlo
                        asum = wp.tile([128, L], BF16, tag="asum", bufs=1,
                                       name=f"as{h}_{w}")
                        nc.vector.tensor_add(
                            asum[:], atile[:, acol(kcs[0], qlo):acol(kcs[0], qhi)],
                            atile[:, acol(kcs[1], qlo):acol(kcs[1], qhi)])
                        for kc in kcs[2:]:
                            nc.vector.tensor_add(
                                asum[:], asum[:],
                                atile[:, acol(kc, qlo):acol(kc, qhi)])
                        dn = pp.tile([128, L], F32, tag="pden", bufs=1,
                                     name=f"dn{h}_{w}")
                        nc.tensor.matmul(dn[:], lhsT=ones_t[:], rhs=asum[:],
                                         start=True, stop=True)
                        rb = wp.tile([128, L], F32, tag="rb", bufs=2,
                                     name=f"rb{h}_{w}")
                        nc.vector.reciprocal_approx_fast(rb[:], dn[:])
                        return rb

                    _ncnt = [0]

                    def ntmp(L, tag):
                        _ncnt[0] += 1
                        return wp.tile([128, L], BF16, tag="nt", bufs=4,
                                       name=f"nt{h}_{_ncnt[0]}")

                    def blend(qlo, n_new):
                        # ao[qlo:qlo+256] += alpha * (n_new - ao[qlo:qlo+256])
                        sl = aoh[:, qlo:qlo + 256]
                        _ncnt[0] += 1
                        t2 = wp.tile([128, 256], BF16, tag="bl2", bufs=1,
                                     name=f"bl2_{h}_{_ncnt[0]}")
                        nc.vector.tensor_sub(t2[:], n_new[:], sl)
                        nc.vector.tensor_mul(t2[:], t2[:], alpha_t[:])
                        nc.vector.tensor_add(sl, sl, t2[:])

                    aoh = ao[:, h * S:(h + 1) * S]

                    # --- window 0 ---
                    score(0, 0, 512, "a0")
                    score(1, 0, 512, "a1")
                    qgroup(0)
                    score(2, 0, 512, "a2")
                    score(3, 0, 512, "a3")
                    qgroup(1)
                    pavA = pp.tile([128, 512], F32, tag="pav", bufs=3, name=f"A{h}")
                    av(pavA, [0, 1], 0, 512)
                    pavB = pp.tile([128, 512], F32, tag="pav", bufs=3, name=f"B{h}")
                    av(pavB, [2, 3], 0, 512)
                    rb0 = denom(0, [0, 1, 2, 3], 0, 512)
                    nA = ntmp(512, "n1")
                    nc.vector.tensor_mul(nA[:], pavA[:], rb0[:])
                    nB = ntmp(512, "n2")
                    nc.vector.tensor_mul(nB[:], pavB[:], rb0[:])
                    nc.vector.tensor_add(aoh[:, 0:512], nA[:], nB[:])
                    qgroup(2)

                    # --- window 1 ---
                    score(4, 256, 768, "a4")
                    score(5, 256, 768, "a5")
                    qgroup(3)
                    score(2, 512, 768, "a2b")
                    score(3, 512, 768, "a3b")
                    pavC1 = pp.tile([128, 512], F32, tag="pav", bufs=3, name=f"C1{h}")
                    av(pavC1, [4, 5], 256, 768)
                    pavC2 = pp.tile([128, 256], F32, tag="pav", bufs=3, name=f"C2{h}")
                    av(pavC2, [2, 3], 512, 768)
                    rb1 = denom(1, [2, 3, 4, 5], 256, 768)
                    nB1 = ntmp(256, "n1")
                    nc.vector.tensor_mul(nB1[:], pavB[:, 256:512], rb1[:, 0:256])
                    nC1 = ntmp(512, "n2")
                    nc.vector.tensor_mul(nC1[:], pavC1[:], rb1[:])
                    n1a = ntmp(256, "n3")
                    nc.vector.tensor_add(n1a[:], nB1[:], nC1[:, 0:256])
                    blend(256, n1a)
                    nC2 = ntmp(256, "n1")
                    nc.vector.tensor_mul(nC2[:], pavC2[:], rb1[:, 256:512])
                    nc.vector.tensor_add(aoh[:, 512:768], nC2[:], nC1[:, 256:512])
                    qgroup(4)

                    # --- window 2 ---
                    score(6, 512, 1024, "a6")
                    score(7, 512, 1024, "a7")
                    qgroup(5)
                    score(4, 768, 1024, "a4b")
                    score(5, 768, 1024, "a5b")
                    pavD = pp.tile([128, 512], F32, tag="pav", bufs=3, name=f"D{h}")
                    av(pavD, [6, 7], 512, 1024)
                    # evacuate Q(h+1) now (qps stopped in qgroup(5)); the ACT
                    # copies and rope DVE work hide under the w2/w3 PE stream
                    if h + 1 < H:
                        qraw = wp.tile([128, S], BF16, tag="qraw", bufs=1,
                                       name=f"qraw{h+1}")
                        for hh in range(2):
                            nc.scalar.copy(qraw[:, hh * 512:(hh + 1) * 512], qps[hh][:])
                        mc_next = rope_mc(qraw)
                    rb2 = denom(2, [4, 5, 6, 7], 512, 1024)
                    nC1b = ntmp(256, "n1")
                    nc.vector.tensor_mul(nC1b[:], pavC1[:, 256:512], rb2[:, 0:256])
                    nD = ntmp(512, "n2")
                    nc.vector.tensor_mul(nD[:], pavD[:], rb2[:])
                    n2a = ntmp(256, "n3")
                    nc.vector.tensor_add(n2a[:], nC1b[:], nD[:, 0:256])
                    blend(512, n2a)
                    pavF = pp.tile([128, 256], F32, tag="pav", bufs=3, name=f"F{h}")
                    av(pavF, [4, 5], 768, 1024)
                    if h + 1 < H:
                        qrt_next = wp.tile([128, S], BF16, tag="qrt", bufs=2,
                                           name=f"qrt{h+1}")
                        rope_rot(qrt_next, qraw, mc_next, f"q{h+1}")
                        qrt_holder[0] = qrt_next
                    nF = ntmp(256, "n1")
                    nc.vector.tensor_mul(nF[:], pavF[:], rb2[:, 256:512])
                    nc.vector.tensor_add(aoh[:, 768:1024], nD[:, 256:512], nF[:])

                    # --- window 3 ---
                    rb3 = denom(3, [6, 7], 768, 1024)
                    n3 = ntmp(256, "n2")
                    nc.vector.tensor_mul(n3[:], pavD[:, 256:512], rb3[:])
                    blend(768, n3)

                    if h == H - 1:
                        # close out O-proj tile 0 with the head-31 term
                        for hh in range(2):
                            nc.tensor.matmul(
                                yps0[hh][:],
                                lhsT=wot_pre[0][:, 31 * 128:32 * 128],
                                rhs=ao[:, 31 * S + hh * 512: 31 * S + (hh + 1) * 512],
                                start=False, stop=True,
                            )
                        yoA0 = wp.tile([128, 512], F32, tag="yoA", bufs=2,
                                       name="yoA_t0")
                        nc.scalar.copy(yoA0[:], yps0[0][:])
                        nc.sync.dma_start(out=y_d[0:128, 0:512], in_=yoA0[:])
                        yoB0 = wp.tile([128, 512], F32, tag="yoB", bufs=2,
                                       name="yoB_t0")
                        nc.vector.tensor_copy(yoB0[:], yps0[1][:])
                        nc.sync.dma_start(out=y_d[0:128, 512:1024], in_=yoB0[:])

              # ---- phase 3: O-projection, transposed (yT = Wo @ aoT) ----
              # psum pool `pp` is closed; open a fresh one for y tiles.
              with tc.tile_pool(name="ops", bufs=1, space="PSUM") as opp:
                    for ot in range(1, 32):
                        if ot in wot_pre:
                            wot = wot_pre[ot]
                        else:
                            wot = wp.tile([128, HD], BF16, tag="wq", bufs=3,
                                          name=f"wo{ot}")
                            nc.sync.dma_start(out=wot[:], in_=wo_d[ot])
                        yps = [opp.tile([128, 512], F32, tag="yps", bufs=6,
                                        name=f"yp{ot}_{hh}") for hh in range(2)]
                        for i in range(NI):
                            for hh in range(2):
                                nc.tensor.matmul(
                                    yps[hh][:],
                                    lhsT=wot[:, i * 128:(i + 1) * 128],
                                    rhs=ao[:, i * S + hh * 512: i * S + (hh + 1) * 512],
                                    start=(i == 0), stop=(i == NI - 1),
                                )
                        yoA = wp.tile([128, 512], F32, tag="yoA", bufs=2,
                                      name=f"yoA{ot}")
                        nc.scalar.copy(yoA[:], yps[0][:])
                        nc.sync.dma_start(
                            out=y_d[ot * 128:(ot + 1) * 128, 0:512], in_=yoA[:])
                        yoB = wp.tile([128, 512], F32, tag="yoB", bufs=2,
                                      name=f"yoB{ot}")
                        nc.vector.tensor_copy(yoB[:], yps[1][:])
                        nc.sync.dma_start(
                            out=y_d[ot * 128:(ot + 1) * 128, 512:1024], in_=yoB[:])
    nc.compile()
    return nc


def prep_inputs(x, Wq, Wk, Wv, Wo):
    """Host-side: transpose/tile/cast so every device DMA is contiguous."""
    bf = ml_dtypes.bfloat16
    xT = np.ascontiguousarray(np.transpose(x, (0, 2, 1))).astype(bf)   # [B,4096,1024]
    # wq[h,p,i*128+c] = Wq[h*128+c, i*128+p]
    wq = np.ascontiguousarray(
        Wq.reshape(H, 128, NI, 128).transpose(0, 3, 2, 1).reshape(H, 128, HD)
    ).astype(bf)
    # wk[p, i*128+c] = Wk[c, i*128+p]
    wk = np.ascontiguousarray(
        Wk.reshape(128, NI, 128).transpose(2, 1, 0).reshape(128, HD)
    ).astype(bf)
    wv = np.ascontiguousarray(
        Wv.reshape(128, NI, 128).transpose(2, 1, 0).reshape(128, HD)
    ).astype(bf)
    # wo[ot, p, i*128+c] = Wo[ot*128+c, i*128+p]
    wo = np.ascontiguousarray(
        Wo.reshape(32, 128, NI, 128).transpose(0, 3, 2, 1).reshape(32, 128, HD)
    ).astype(bf)
    cos, sin = _rope_cache_np(S, DH)
    cosT = np.ascontiguousarray(cos.T).astype(bf)                      # [128,1024]
    sinS = np.ascontiguousarray(sin.T).astype(bf)
    rotm = np.zeros((128, 128), dtype=np.float32)
    rotm[np.arange(64) + 64, np.arange(64)] = -1.0
    rotm[np.arange(64), np.arange(64) + 64] = 1.0
    rotm = rotm.astype(bf)
    alphaB = np.tile(
        np.linspace(0.0, 1.0, 256, dtype=np.float32)[None, :], (128, 1)
    ).astype(bf)
    ident = np.eye(128, dtype=np.float32).astype(bf)
    shared = dict(wq=wq, wk=wk, wv=wv, wo=wo, cosT=cosT, sinS=sinS, alphaB=alphaB,
                  rotm=rotm, ident=ident)
    in_maps = [dict(xT=xT[b], **shared) for b in range(B)]
    return in_maps


def kernel(x, Wq, Wk, Wv, Wo):
    if "nc" not in _CACHE:
        _CACHE["nc"] = build_nc()
    nc = _CACHE["nc"]
    in_maps = prep_inputs(
        np.asarray(x, dtype=np.float32),
        np.asarray(Wq, dtype=np.float32),
        np.asarray(Wk, dtype=np.float32),
        np.asarray(Wv, dtype=np.float32),
        np.asarray(Wo, dtype=np.float32),
    )
    res = run_bass_kernel_spmd(nc, in_maps, core_ids=list(range(B)))
    out = np.stack(
        [np.ascontiguousarray(np.asarray(res.results[b]["y"]).T) for b in range(B)],
        axis=0,
    )
    return out.astype(np.float32)


if __name__ == "__main__":
    rng = np.random.default_rng(0)
    x = rng.standard_normal((B, S, HD), dtype=np.float32)
    Wq = (rng.standard_normal((HD, HD), dtype=np.float32) * 0.02)
    Wk = (rng.standard_normal((DH, HD), dtype=np.float32) * 0.02)
    Wv = (rng.standard_normal((DH, HD), dtype=np.float32) * 0.02)
    Wo = (rng.standard_normal((HD, HD), dtype=np.float32) * 0.02)
    y = kernel(x=x, Wq=Wq, Wk=Wk, Wv=Wv, Wo=Wo)
    print("out", y.shape, y.dtype, float(np.abs(y).mean()))


# revision 7
# speedup vs baseline: 1.2503x; 1.0017x over previous
"""Trainium2 Bass kernel for nn_AdvancedFastMQA — v2.

Data-parallel over batch B=8 across 8 NeuronCores. Transposed dataflow
(no on-device transposes except V). Per-core PE-cycle cuts vs v1:

 - Sliding-window overlap sharing: score tiles and attention@V partial
   sums for the k-chunk regions shared by adjacent windows are computed
   once (40 unit tiles instead of 52 for both scores and AV). Window
   outputs are assembled from 2-chunk PSUM partials (A,B,C1,C2,D,F) with
   cheap DVE combines.
 - Denominator: instead of M=1 ones-vector matmuls per k-chunk (same
   streaming cost as scores, zero useful flops), the k-chunk sigmoid
   tiles are summed on DVE and a single ones[128x128] matmul per window
   produces the partition-sum already broadcast across 128 partitions
   (also kills the gpsimd broadcast).
 - O-projection computed transposed: yT[o,t] = sum_i wo_tile[i].T @ ao_i
   with the weight stationary for 2 matmuls each, PSUM double-buffered;
   host transposes the [4096,1024] result back.
 - Q-projection of head h+1 is interleaved into attention of head h so
   the PE never waits on ACT sigmoids.

Windows (S=1024, window=512, stride 256):
  w0: k,q in [0,512); w1: k,q [256,768); w2: k,q [512,1024);
  w3: k,q [768,1024). Blend regions: [256,512) w0/w1, [512,768) w1/w2,
  [768,1024) w2/w3, alpha = linspace(0,1,256).

AV partial-sum plan (per head, PSUM tiles, kc = 128-wide k chunks):
  A  = kc0+kc1 over q[0:512)      B  = kc2+kc3 over q[0:512)
  C1 = kc4+kc5 over q[256:768)    C2 = kc2+kc3 over q[512:768)
  D  = kc6+kc7 over q[512:1024)   F  = kc4+kc5 over q[768:1024)
  u0 = A+B; u1 = B[256:512)+C1 | C2+C1[512:768); u2 = C1+D | D+F;
  u3 = D[768:1024).
"""

import sys

for _p in ("/opt/trn_rl_repo", "/opt/pypackages"):
    if _p not in sys.path:
        sys.path.append(_p)

import numpy as np
import ml_dtypes

import concourse.bacc as bacc
import concourse.tile as tile
import concourse.mybir as mybir
import concourse.bass_isa as bass_isa
from concourse.bass_utils import run_bass_kernel_spmd

BF16 = mybir.dt.bfloat16
F32 = mybir.dt.float32
AF = mybir.ActivationFunctionType

B, S, HD = 8, 1024, 4096
H, DH = 32, 128
WINDOW = 512
SCALE = 1.0 / float(np.sqrt(DH))
ROPE_BASE = 10000.0
NI = HD // 128          # 32 contraction chunks
NT = S // 128           # 8 token chunks

# a-tile (sigmoid) layout: per kc the union of q-ranges that need it.
A_QLO = [0, 0, 0, 0, 256, 256, 512, 512]
A_W = [512, 512, 768, 768, 768, 768, 512, 512]
A_OFF = [0, 512, 1024, 1792, 2560, 3328, 4096, 4608]
A_TOT = 5120

_CACHE = {}


def _rope_cache_np(S_, D_, base=ROPE_BASE):
    inv_freq = 1.0 / (base ** (np.arange(0, D_, 2, dtype=np.float32) / D_))
    t = np.arange(S_, dtype=np.float32)
    f = np.outer(t, inv_freq)
    cos = np.zeros((S_, D_), dtype=np.float32)
    sin = np.zeros((S_, D_), dtype=np.float32)
    cos[:, 0::2] = np.cos(f)
    cos[:, 1::2] = np.cos(f)
    sin[:, 0::2] = np.sin(f)
    sin[:, 1::2] = np.sin(f)
    return cos, sin


def build_nc():
    nc = bacc.Bacc("TRN2", debug=False, target_bir_lowering=False)

    xT_d = nc.dram_tensor("xT", [HD, S], BF16, kind="ExternalInput").ap()
    wq_d = nc.dram_tensor("wq", [H, 128, HD], BF16, kind="ExternalInput").ap()
    wk_d = nc.dram_tensor("wk", [128, HD], BF16, kind="ExternalInput").ap()
    wv_d = nc.dram_tensor("wv", [128, HD], BF16, kind="ExternalInput").ap()
    wo_d = nc.dram_tensor("wo", [32, 128, HD], BF16, kind="ExternalInput").ap()
    cos_d = nc.dram_tensor("cosT", [128, S], BF16, kind="ExternalInput").ap()
    sin_d = nc.dram_tensor("sinS", [128, S], BF16, kind="ExternalInput").ap()
    alpha_d = nc.dram_tensor("alphaB", [128, 256], BF16, kind="ExternalInput").ap()
    rotm_d = nc.dram_tensor("rotm", [128, 128], BF16, kind="ExternalInput").ap()
    iden_d = nc.dram_tensor("ident", [128, 128], BF16, kind="ExternalInput").ap()
    y_d = nc.dram_tensor("y", [HD, S], F32, kind="ExternalOutput").ap()

    with tile.TileContext(nc) as tc:
        with tc.tile_pool(name="consts", bufs=1) as cp:
            xt = cp.tile([128, NI * S], BF16)              # 64KB/part
            cos_t = cp.tile([128, S], BF16)
            sin_t = cp.tile([128, S], BF16)
            alpha_t = cp.tile([128, 256], BF16)
            ones_t = cp.tile([128, 128], BF16)
            nc.vector.memset(ones_t[:], 1.0)
            rotm_t = cp.tile([128, 128], BF16)
            iden_t = cp.tile([128, 128], BF16)

            kr_t = cp.tile([128, S], BF16)                 # roped K
            v_all = cp.tile([128, NT * 128], BF16)         # V as 8 lhsT tiles
            ao = cp.tile([128, H * S], BF16)               # attention out, 64KB/part

            with tc.tile_pool(name="work", bufs=1) as wp:
              with tc.tile_pool(name="ps", bufs=1, space="PSUM") as pp:

                def rope_mc(src):
                    mc = wp.tile([128, S], BF16, tag="rope_mc", bufs=1)
                    nc.vector.tensor_mul(mc[:], src[:], cos_t[:])
                    return mc

                def rope_rot(dst, src, mc, tag):
                    for rh in range(2):
                        rp = pp.tile([128, 512], F32, tag="pden", bufs=1,
                                     name=f"rot_{tag}_{rh}")
                        nc.tensor.matmul(
                            rp[:], lhsT=rotm_t[:],
                            rhs=src[:, rh * 512:(rh + 1) * 512],
                            start=True, stop=True,
                        )
                        ms = wp.tile([128, 512], BF16, tag="rope_ms", bufs=1)
                        nc.vector.tensor_mul(ms[:], rp[:], sin_t[:, rh * 512:(rh + 1) * 512])
                        nc.vector.tensor_add(
                            dst[:, rh * 512:(rh + 1) * 512],
                            mc[:, rh * 512:(rh + 1) * 512], ms[:],
                        )

                def rope(dst, src, tag):
                    # dst = src*cos + rotate_half(src)*sin; rotate via PE
                    rope_rot(dst, src, rope_mc(src), tag)

                # ---- phase 1: interleaved K / VT / Q-head0 projections ----
                # DMA order matters: the first matmuls need the weights and
                # x chunk 0, so those go first; bulk x and rope consts after.
                wk_t = wp.tile([128, HD], BF16, tag="wq", bufs=3, name="wk")
                wv_t = wp.tile([128, HD], BF16, tag="wq", bufs=3, name="wv")
                wq0_t = wp.tile([128, HD], BF16, tag="wq", bufs=3, name="wq0")
                def wpiece(p):
                    c0 = p * 1024
                    nc.sync.dma_start(out=wk_t[:, c0:c0 + 1024],
                                      in_=wk_d[:, c0:c0 + 1024])
                    nc.sync.dma_start(out=wv_t[:, c0:c0 + 1024],
                                      in_=wv_d[:, c0:c0 + 1024])
                    nc.sync.dma_start(out=wq0_t[:, c0:c0 + 1024],
                                      in_=wq_d[0, :, c0:c0 + 1024])

                # DMA packets drain the queue ~in order, so emit transfers in
                # exact consumption order: weight piece p just before the x
                # chunks that use it.
                for nm, wt_, wd_ in (("wk", None, None),):
                    pass
                nc.sync.dma_start(out=wk_t[:, 0:256], in_=wk_d[:, 0:256])
                nc.sync.dma_start(out=wv_t[:, 0:256], in_=wv_d[:, 0:256])
                nc.sync.dma_start(out=wq0_t[:, 0:256], in_=wq_d[0, :, 0:256])
                for i in range(NI):
                    if i == 1:
                        nc.sync.dma_start(out=wk_t[:, 256:1024],
                                          in_=wk_d[:, 256:1024])
                        nc.sync.dma_start(out=wv_t[:, 256:1024],
                                          in_=wv_d[:, 256:1024])
                        nc.sync.dma_start(out=wq0_t[:, 256:1024],
                                          in_=wq_d[0, :, 256:1024])
                    nc.sync.dma_start(
                        out=xt[:, i * S:(i + 1) * S], in_=xT_d[i * 128:(i + 1) * 128, :]
                    )
                    if i == 5:
                        wpiece(1)
                    if i == 13:
                        wpiece(2)
                    if i == 21:
                        wpiece(3)
                    if i == 8:
                        nc.sync.dma_start(out=cos_t[:], in_=cos_d[:])
                        nc.sync.dma_start(out=sin_t[:], in_=sin_d[:])
                    if i == 10:
                        nc.sync.dma_start(out=alpha_t[:], in_=alpha_d[:])
                        nc.sync.dma_start(out=rotm_t[:], in_=rotm_d[:])
                        nc.sync.dma_start(out=iden_t[:], in_=iden_d[:])
                kps = [pp.tile([128, 512], F32, tag="pscore", bufs=2, name=f"kp{hh}") for hh in range(2)]
                vps = [pp.tile([128, 512], F32, tag="pav", bufs=3, name=f"vp{hh}") for hh in range(2)]
                qps0 = [pp.tile([128, 512], F32, tag="pproj", bufs=2, name=f"qp0{hh}") for hh in range(2)]
                for i in range(NI):
                    st_ = (i == 0)
                    sp_ = (i == NI - 1)
                    for hh in range(2):
                        rhs = xt[:, i * S + hh * 512: i * S + (hh + 1) * 512]
                        nc.tensor.matmul(kps[hh][:], lhsT=wk_t[:, i * 128:(i + 1) * 128],
                                         rhs=rhs, start=st_, stop=sp_)
                        nc.tensor.matmul(vps[hh][:], lhsT=wv_t[:, i * 128:(i + 1) * 128],
                                         rhs=rhs, start=st_, stop=sp_)
                        nc.tensor.matmul(qps0[hh][:], lhsT=wq0_t[:, i * 128:(i + 1) * 128],
                                         rhs=rhs, start=st_, stop=sp_)
                kraw = wp.tile([128, S], BF16, tag="yoA", bufs=2, name="kraw")
                vtraw = wp.tile([128, S], BF16, tag="qrt", bufs=2)
                qraw0 = wp.tile([128, S], BF16, tag="qraw", bufs=1)
                for hh in range(2):
                    nc.scalar.copy(kraw[:, hh * 512:(hh + 1) * 512], kps[hh][:])
                    nc.scalar.copy(qraw0[:, hh * 512:(hh + 1) * 512], qps0[hh][:])
                    nc.vector.tensor_copy(vtraw[:, hh * 512:(hh + 1) * 512], vps[hh][:])
                rope(kr_t, kraw, "k")
                for t in range(NT):
                    tp = pp.tile([128, 128], BF16, tag="pscore", bufs=2, name=f"vtp{t}")
                    nc.tensor.transpose(tp[:], vtraw[:, t * 128:(t + 1) * 128], iden_t[:])
                    nc.scalar.copy(v_all[:, t * 128:(t + 1) * 128], tp[:])
                qrt0 = wp.tile([128, S], BF16, tag="qrt", bufs=2)
                rope(qrt0, qraw0, "q0")

                # ---- phase 2: per-head attention + interleaved Q proj(h+1) ----
                def acol(kc, q):
                    return A_OFF[kc] + q - A_QLO[kc]

                qrt_holder = [qrt0]
                wot_pre = {}
                yps0 = None
                for h in range(H):
                    qrt = qrt_holder[0]

                    # Q projection emitters for head h+1, in 8 groups of 4 i's
                    if h + 1 < H:
                        wq_t = wp.tile([128, HD], BF16, tag="wq", bufs=3)
                        nc.sync.dma_start(out=wq_t[:], in_=wq_d[h + 1])
                        qps = [pp.tile([128, 512], F32, tag="pproj", bufs=2,
                                       name=f"qp{h+1}_{hh}") for hh in range(2)]

                        QG = [(0, 5), (5, 10), (10, 15), (15, 20), (20, 26),
                              (26, 30), (30, 32)]

                        def qgroup(g, wq_t=wq_t, qps=qps):
                            for i in range(*QG[g]):
                                for hh in range(2):
                                    nc.tensor.matmul(
                                        qps[hh][:],
                                        lhsT=wq_t[:, i * 128:(i + 1) * 128],
                                        rhs=xt[:, i * S + hh * 512: i * S + (hh + 1) * 512],
                                        start=(i == 0), stop=(i == NI - 1),
                                    )
                    else:
                        # last head: no Q projection to interleave — fill the
                        # PE with O-proj tile 0 (heads 0..30 partial sums) in
                        # the idle pproj PSUM banks instead.
                        QG = [(0, 5), (5, 10), (10, 15), (15, 20), (20, 26),
                              (26, 30), (30, 31)]
                        w0t = wot_pre[0]
                        yps0 = [pp.tile([128, 512], F32, tag="pproj", bufs=2,
                                        name=f"y0_{hh}") for hh in range(2)]

                        def qgroup(g, w0t=w0t, yps0=yps0):
                            for i in range(*QG[g]):
                                if i > 30:
                                    continue
                                for hh in range(2):
                                    nc.tensor.matmul(
                                        yps0[hh][:],
                                        lhsT=w0t[:, i * 128:(i + 1) * 128],
                                        rhs=ao[:, i * S + hh * 512: i * S + (hh + 1) * 512],
                                        start=(i == 0), stop=False,
                                    )

                    if h == H - 2:
                        # prefetch the first O-proj weight slabs (allocated
                        # after wq31 so the wq-slot rotation stays acyclic);
                        # their DMAs overlap the last two heads' attention
                        for ot in range(3):
                            w_pre = wp.tile([128, HD], BF16, tag="wq", bufs=3,
                                            name=f"wo{ot}")
                            nc.sync.dma_start(out=w_pre[:], in_=wo_d[ot])
                            wot_pre[ot] = w_pre

                    atile = wp.tile([128, A_TOT], BF16, tag="attn", bufs=2)

                    def score(kc, qlo, qhi, tag):
                        L = qhi - qlo
                        sp = pp.tile([128, L], F32, tag="pscore", bufs=2,
                                     name=f"sc{h}_{tag}")
                        nc.tensor.matmul(
                            sp[:], lhsT=kr_t[:, kc * 128:(kc + 1) * 128],
                            rhs=qrt[:, qlo:qhi], start=True, stop=True,
                        )
                        nc.scalar.activation(
                            atile[:, acol(kc, qlo):acol(kc, qhi)], sp[:],
                            AF.Sigmoid, scale=SCALE,
                        )

                    def av(dst, kcs, qlo, qhi):
                        # dst += sum_kc V[kc].T @ a[kc, qlo:qhi]
                        for j, kc in enumerate(kcs):
                            nc.tensor.matmul(
                                dst[:],
                                lhsT=v_all[:, kc * 128:(kc + 1) * 128],
                                rhs=atile[:, acol(kc, qlo):acol(kc, qhi)],
                                start=(j == 0), stop=(j == len(kcs) - 1),
                            )

                    def denom(w, kcs, qlo, qhi):
                        # rb = 1 / sum_k sigmoid over window (broadcast on 128 parts)
                        L = qhi - qlo
                        asum = wp.tile([128, L], BF16, tag="asum", bufs=1,
                                       name=f"as{h}_{w}")
                        nc.vector.tensor_add(
                            asum[:], atile[:, acol(kcs[0], qlo):acol(kcs[0], qhi)],
                            atile[:, acol(kcs[1], qlo):acol(kcs[1], qhi)])
                        for kc in kcs[2:]:
                            nc.vector.tensor_add(
                                asum[:], asum[:],
                                atile[:, acol(kc, qlo):acol(kc, qhi)])
                        dn = pp.tile([128, L], F32, tag="pden", bufs=1,
                                     name=f"dn{h}_{w}")
                        nc.tensor.matmul(dn[:], lhsT=ones_t[:], rhs=asum[:],
                                         start=True, stop=True)
                        rb = wp.tile([128, L], F32, tag="rb", bufs=2,
                                     name=f"rb{h}_{w}")
                        nc.vector.reciprocal_approx_fast(rb[:], dn[:])
                        return rb

                    _ncnt = [0]

                    def ntmp(L, tag):
                        _ncnt[0] += 1
                        return wp.tile([128, L], BF16, tag="nt", bufs=4,
                                       name=f"nt{h}_{_ncnt[0]}")

                    def blend(qlo, n_new):
                        # ao[qlo:qlo+256] += alpha * (n_new - ao[qlo:qlo+256])
                        sl = aoh[:, qlo:qlo + 256]
                        _ncnt[0] += 1
                        t2 = wp.tile([128, 256], BF16, tag="bl2", bufs=1,
                                     name=f"bl2_{h}_{_ncnt[0]}")
                        nc.vector.tensor_sub(t2[:], n_new[:], sl)
                        nc.vector.tensor_mul(t2[:], t2[:], alpha_t[:])
                        nc.vector.tensor_add(sl, sl, t2[:])

                    aoh = ao[:, h * S:(h + 1) * S]

                    # --- window 0 ---
                    score(0, 0, 512, "a0")
                    score(1, 0, 512, "a1")
                    qgroup(0)
                    score(2, 0, 512, "a2")
                    score(3, 0, 512, "a3")
                    qgroup(1)
                    pavA = pp.tile([128, 512], F32, tag="pav", bufs=3, name=f"A{h}")
                    av(pavA, [0, 1], 0, 512)
                    pavB = pp.tile([128, 512], F32, tag="pav", bufs=3, name=f"B{h}")
                    av(pavB, [2, 3], 0, 512)
                    rb0 = denom(0, [0, 1, 2, 3], 0, 512)
                    nA = ntmp(512, "n1")
                    nc.vector.tensor_mul(nA[:], pavA[:], rb0[:])
                    nB = ntmp(512, "n2")
                    nc.vector.tensor_mul(nB[:], pavB[:], rb0[:])
                    nc.vector.tensor_add(aoh[:, 0:512], nA[:], nB[:])
                    qgroup(2)

                    # --- window 1 ---
                    score(4, 256, 768, "a4")
                    score(5, 256, 768, "a5")
                    qgroup(3)
                    score(2, 512, 768, "a2b")
                    score(3, 512, 768, "a3b")
                    pavC1 = pp.tile([128, 512], F32, tag="pav", bufs=3, name=f"C1{h}")
                    av(pavC1, [4, 5], 256, 768)
                    pavC2 = pp.tile([128, 256], F32, tag="pav", bufs=3, name=f"C2{h}")
                    av(pavC2, [2, 3], 512, 768)
                    rb1 = denom(1, [2, 3, 4, 5], 256, 768)
                    nB1 = ntmp(256, "n1")
                    nc.vector.tensor_mul(nB1[:], pavB[:, 256:512], rb1[:, 0:256])
                    nC1 = ntmp(512, "n2")
                    nc.vector.tensor_mul(nC1[:], pavC1[:], rb1[:])
                    n1a = ntmp(256, "n3")
                    nc.vector.tensor_add(n1a[:], nB1[:], nC1[:, 0:256])
                    blend(256, n1a)
                    nC2 = ntmp(256, "n1")
                    nc.vector.tensor_mul(nC2[:], pavC2[:], rb1[:, 256:512])
                    nc.vector.tensor_add(aoh[:, 512:768], nC2[:], nC1[:, 256:512])
                    qgroup(4)

                    # --- window 2 --- (F allocated before D so the pav
                    # slot the next head's early groups rotate into frees
                    # at w2-end rather than at w3-end)
                    score(6, 512, 1024, "a6")
                    score(7, 512, 1024, "a7")
                    qgroup(5)
                    score(4, 768, 1024, "a4b")
                    score(5, 768, 1024, "a5b")
                    pavF = pp.tile([128, 256], F32, tag="pav", bufs=3, name=f"F{h}")
                    av(pavF, [4, 5], 768, 1024)
                    rb2 = denom(2, [4, 5, 6, 7], 512, 1024)
                    nC1b = ntmp(256, "n1")
                    nc.vector.tensor_mul(nC1b[:], pavC1[:, 256:512], rb2[:, 0:256])
                    # window-3 denominator sum now (slot freed by dn2 matmul;
                    # keeps the late den3 matmul off the DVE critical path)
                    asum3 = wp.tile([128, 256], BF16, tag="asum", bufs=1,
                                    name=f"as{h}_3")
                    nc.vector.tensor_add(
                        asum3[:], atile[:, acol(6, 768):acol(6, 1024)],
                        atile[:, acol(7, 768):acol(7, 1024)])
                    qgroup(6)
                    if h + 1 < H:
                        qraw = wp.tile([128, S], BF16, tag="qraw", bufs=1,
                                       name=f"qraw{h+1}")
                        for hh in range(2):
                            nc.scalar.copy(qraw[:, hh * 512:(hh + 1) * 512], qps[hh][:])
                        mc_next = rope_mc(qraw)
                    pavD = pp.tile([128, 512], F32, tag="pav", bufs=3, name=f"D{h}")
                    av(pavD, [6, 7], 512, 1024)
                    if h + 1 < H:
                        qrt_next = wp.tile([128, S], BF16, tag="qrt", bufs=2,
                                           name=f"qrt{h+1}")
                        rope_rot(qrt_next, qraw, mc_next, f"q{h+1}")
                        qrt_holder[0] = qrt_next
                    nD = ntmp(512, "n2")
                    nc.vector.tensor_mul(nD[:], pavD[:], rb2[:])
                    n2a = ntmp(256, "n3")
                    nc.vector.tensor_add(n2a[:], nC1b[:], nD[:, 0:256])
                    blend(512, n2a)
                    nF = ntmp(256, "n1")
                    nc.vector.tensor_mul(nF[:], pavF[:], rb2[:, 256:512])
                    nc.vector.tensor_add(aoh[:, 768:1024], nD[:, 256:512], nF[:])

                    # --- window 3 ---
                    dn3 = pp.tile([128, 256], F32, tag="pden", bufs=1,
                                  name=f"dn{h}_3")
                    nc.tensor.matmul(dn3[:], lhsT=ones_t[:], rhs=asum3[:],
                                     start=True, stop=True)
                    rb3 = wp.tile([128, 256], F32, tag="rb", bufs=2,
                                  name=f"rb{h}_3")
                    nc.vector.reciprocal_approx_fast(rb3[:], dn3[:])
                    n3 = ntmp(256, "n2")
                    nc.vector.tensor_mul(n3[:], pavD[:, 256:512], rb3[:])
                    blend(768, n3)

                    if h == H - 1:
                        # close out O-proj tile 0 with the head-31 term
                        for hh in range(2):
                            nc.tensor.matmul(
                                yps0[hh][:],
                                lhsT=wot_pre[0][:, 31 * 128:32 * 128],
                                rhs=ao[:, 31 * S + hh * 512: 31 * S + (hh + 1) * 512],
                                start=False, stop=True,
                            )
                        yoA0 = wp.tile([128, 512], F32, tag="yoA", bufs=2,
                                       name="yoA_t0")
                        nc.scalar.copy(yoA0[:], yps0[0][:])
                        nc.sync.dma_start(out=y_d[0:128, 0:512], in_=yoA0[:])
                        yoB0 = wp.tile([128, 512], F32, tag="yoB", bufs=2,
                                       name="yoB_t0")
                        nc.vector.tensor_copy(yoB0[:], yps0[1][:])
                        nc.sync.dma_start(out=y_d[0:128, 512:1024], in_=yoB0[:])

              # ---- phase 3: O-projection, transposed (yT = Wo @ aoT) ----
              # psum pool `pp` is closed; open a fresh one for y tiles.
              with tc.tile_pool(name="ops", bufs=1, space="PSUM") as opp:
                    for ot in range(1, 32):
                        if ot in wot_pre:
                            wot = wot_pre[ot]
                        else:
                            wot = wp.tile([128, HD], BF16, tag="wq", bufs=3,
                                          name=f"wo{ot}")
                            nc.sync.dma_start(out=wot[:], in_=wo_d[ot])
                        yps = [opp.tile([128, 512], F32, tag="yps", bufs=6,
                                        name=f"yp{ot}_{hh}") for hh in range(2)]
                        for i in range(NI):
                            for hh in range(2):
                                nc.tensor.matmul(
                                    yps[hh][:],
                                    lhsT=wot[:, i * 128:(i + 1) * 128],
                                    rhs=ao[:, i * S + hh * 512: i * S + (hh + 1) * 512],
                                    start=(i == 0), stop=(i == NI - 1),
                                )
                        yoA = wp.tile([128, 512], F32, tag="yoA", bufs=2,
                                      name=f"yoA{ot}")
                        nc.scalar.copy(yoA[:], yps[0][:])
                        nc.sync.dma_start(
                            out=y_d[ot * 128:(ot + 1) * 128, 0:512], in_=yoA[:])
                        yoB = wp.tile([128, 512], F32, tag="yoB", bufs=2,
                                      name=f"yoB{ot}")
                        nc.vector.tensor_copy(yoB[:], yps[1][:])
                        nc.sync.dma_start(
                            out=y_d[ot * 128:(ot + 1) * 128, 512:1024], in_=yoB[:])
    nc.compile()
    return nc


def prep_inputs(x, Wq, Wk, Wv, Wo):
    """Host-side: transpose/tile/cast so every device DMA is contiguous."""
    bf = ml_dtypes.bfloat16
    xT = np.ascontiguousarray(np.transpose(x, (0, 2, 1))).astype(bf)   # [B,4096,1024]
    # wq[h,p,i*128+c] = Wq[h*128+c, i*128+p]
    wq = np.ascontiguousarray(
        Wq.reshape(H, 128, NI, 128).transpose(0, 3, 2, 1).reshape(H, 128, HD)
    ).astype(bf)
    # wk[p, i*128+c] = Wk[c, i*128+p]
    wk = np.ascontiguousarray(
        Wk.reshape(128, NI, 128).transpose(2, 1, 0).reshape(128, HD)
    ).astype(bf)
    wv = np.ascontiguousarray(
        Wv.reshape(128, NI, 128).transpose(2, 1, 0).reshape(128, HD)
    ).astype(bf)
    # wo[ot, p, i*128+c] = Wo[ot*128+c, i*128+p]
    wo = np.ascontiguousarray(
        Wo.reshape(32, 128, NI, 128).transpose(0, 3, 2, 1).reshape(32, 128, HD)
    ).astype(bf)
    cos, sin = _rope_cache_np(S, DH)
    cosT = np.ascontiguousarray(cos.T).astype(bf)                      # [128,1024]
    sinS = np.ascontiguousarray(sin.T).astype(bf)
    rotm = np.zeros((128, 128), dtype=np.float32)
    rotm[np.arange(64) + 64, np.arange(64)] = -1.0
    rotm[np.arange(64), np.arange(64) + 64] = 1.0
    rotm = rotm.astype(bf)
    alphaB = np.tile(
        np.linspace(0.0, 1.0, 256, dtype=np.float32)[None, :], (128, 1)
    ).astype(bf)
    ident = np.eye(128, dtype=np.float32).astype(bf)
    shared = dict(wq=wq, wk=wk, wv=wv, wo=wo, cosT=cosT, sinS=sinS, alphaB=alphaB,
                  rotm=rotm, ident=ident)
    in_maps = [dict(xT=xT[b], **shared) for b in range(B)]
    return in_maps


def kernel(x, Wq, Wk, Wv, Wo):
    if "nc" not in _CACHE:
        _CACHE["nc"] = build_nc()
    nc = _CACHE["nc"]
    in_maps = prep_inputs(
        np.asarray(x, dtype=np.float32),
        np.asarray(Wq, dtype=np.float32),
        np.asarray(Wk, dtype=np.float32),
        np.asarray(Wv, dtype=np.float32),
        np.asarray(Wo, dtype=np.float32),
    )
    res = run_bass_kernel_spmd(nc, in_maps, core_ids=list(range(B)))
    out = np.stack(
        [np.ascontiguousarray(np.asarray(res.results[b]["y"]).T) for b in range(B)],
        axis=0,
    )
    return out.astype(np.float32)


if __name__ == "__main__":
    rng = np.random.default_rng(0)
    x = rng.standard_normal((B, S, HD), dtype=np.float32)
    Wq = (rng.standard_normal((HD, HD), dtype=np.float32) * 0.02)
    Wk = (rng.standard_normal((DH, HD), dtype=np.float32) * 0.02)
    Wv = (rng.standard_normal((DH, HD), dtype=np.float32) * 0.02)
    Wo = (rng.standard_normal((HD, HD), dtype=np.float32) * 0.02)
    y = kernel(x=x, Wq=Wq, Wk=Wk, Wv=Wv, Wo=Wo)
    print("out", y.shape, y.dtype, float(np.abs(y).mean()))


# revision 8
# speedup vs baseline: 1.2576x; 1.0058x over previous
"""Trainium2 Bass kernel for nn_AdvancedFastMQA — v2.

Data-parallel over batch B=8 across 8 NeuronCores. Transposed dataflow
(no on-device transposes except V). Per-core PE-cycle cuts vs v1:

 - Sliding-window overlap sharing: score tiles and attention@V partial
   sums for the k-chunk regions shared by adjacent windows are computed
   once (40 unit tiles instead of 52 for both scores and AV). Window
   outputs are assembled from 2-chunk PSUM partials (A,B,C1,C2,D,F) with
   cheap DVE combines.
 - Denominator: instead of M=1 ones-vector matmuls per k-chunk (same
   streaming cost as scores, zero useful flops), the k-chunk sigmoid
   tiles are summed on DVE and a single ones[128x128] matmul per window
   produces the partition-sum already broadcast across 128 partitions
   (also kills the gpsimd broadcast).
 - O-projection computed transposed: yT[o,t] = sum_i wo_tile[i].T @ ao_i
   with the weight stationary for 2 matmuls each, PSUM double-buffered;
   host transposes the [4096,1024] result back.
 - Q-projection of head h+1 is interleaved into attention of head h so
   the PE never waits on ACT sigmoids.

Windows (S=1024, window=512, stride 256):
  w0: k,q in [0,512); w1: k,q [256,768); w2: k,q [512,1024);
  w3: k,q [768,1024). Blend regions: [256,512) w0/w1, [512,768) w1/w2,
  [768,1024) w2/w3, alpha = linspace(0,1,256).

AV partial-sum plan (per head, PSUM tiles, kc = 128-wide k chunks):
  A  = kc0+kc1 over q[0:512)      B  = kc2+kc3 over q[0:512)
  C1 = kc4+kc5 over q[256:768)    C2 = kc2+kc3 over q[512:768)
  D  = kc6+kc7 over q[512:1024)   F  = kc4+kc5 over q[768:1024)
  u0 = A+B; u1 = B[256:512)+C1 | C2+C1[512:768); u2 = C1+D | D+F;
  u3 = D[768:1024).
"""

import sys

for _p in ("/opt/trn_rl_repo", "/opt/pypackages"):
    if _p not in sys.path:
        sys.path.append(_p)

import numpy as np
import ml_dtypes

import concourse.bacc as bacc
import concourse.tile as tile
import concourse.mybir as mybir
import concourse.bass_isa as bass_isa
from concourse.bass_utils import run_bass_kernel_spmd

BF16 = mybir.dt.bfloat16
F32 = mybir.dt.float32
AF = mybir.ActivationFunctionType

B, S, HD = 8, 1024, 4096
H, DH = 32, 128
WINDOW = 512
SCALE = 1.0 / float(np.sqrt(DH))
ROPE_BASE = 10000.0
NI = HD // 128          # 32 contraction chunks
NT = S // 128           # 8 token chunks

# a-tile (sigmoid) layout: per kc the union of q-ranges that need it.
A_QLO = [0, 0, 0, 0, 256, 256, 512, 512]
A_W = [512, 512, 768, 768, 768, 768, 512, 512]
A_OFF = [0, 512, 1024, 1792, 2560, 3328, 4096, 4608]
A_TOT = 5120

_CACHE = {}


def _rope_cache_np(S_, D_, base=ROPE_BASE):
    inv_freq = 1.0 / (base ** (np.arange(0, D_, 2, dtype=np.float32) / D_))
    t = np.arange(S_, dtype=np.float32)
    f = np.outer(t, inv_freq)
    cos = np.zeros((S_, D_), dtype=np.float32)
    sin = np.zeros((S_, D_), dtype=np.float32)
    cos[:, 0::2] = np.cos(f)
    cos[:, 1::2] = np.cos(f)
    sin[:, 0::2] = np.sin(f)
    sin[:, 1::2] = np.sin(f)
    return cos, sin


def build_nc():
    nc = bacc.Bacc("TRN2", debug=False, target_bir_lowering=False)

    xT_d = nc.dram_tensor("xT", [HD, S], BF16, kind="ExternalInput").ap()
    wq_d = nc.dram_tensor("wq", [H, 128, HD], BF16, kind="ExternalInput").ap()
    wk_d = nc.dram_tensor("wk", [128, HD], BF16, kind="ExternalInput").ap()
    wv_d = nc.dram_tensor("wv", [128, HD], BF16, kind="ExternalInput").ap()
    wo_d = nc.dram_tensor("wo", [32, 128, HD], BF16, kind="ExternalInput").ap()
    cos_d = nc.dram_tensor("cosT", [128, S], BF16, kind="ExternalInput").ap()
    sin_d = nc.dram_tensor("sinS", [128, S], BF16, kind="ExternalInput").ap()
    alpha_d = nc.dram_tensor("alphaB", [128, 256], BF16, kind="ExternalInput").ap()
    rotm_d = nc.dram_tensor("rotm", [128, 128], BF16, kind="ExternalInput").ap()
    iden_d = nc.dram_tensor("ident", [128, 128], BF16, kind="ExternalInput").ap()
    y_d = nc.dram_tensor("y", [HD, S], F32, kind="ExternalOutput").ap()

    with tile.TileContext(nc) as tc:
        with tc.tile_pool(name="consts", bufs=1) as cp:
            xt = cp.tile([128, NI * S], BF16)              # 64KB/part
            cos_t = cp.tile([128, S], BF16)
            sin_t = cp.tile([128, S], BF16)
            alpha_t = cp.tile([128, 256], BF16)
            ones_t = cp.tile([128, 128], BF16)
            nc.vector.memset(ones_t[:], 1.0)
            rotm_t = cp.tile([128, 128], BF16)
            iden_t = cp.tile([128, 128], BF16)

            kr_t = cp.tile([128, S], BF16)                 # roped K
            v_all = cp.tile([128, NT * 128], BF16)         # V as 8 lhsT tiles
            ao = cp.tile([128, H * S], BF16)               # attention out, 64KB/part

            with tc.tile_pool(name="work", bufs=1) as wp:
              with tc.tile_pool(name="ps", bufs=1, space="PSUM") as pp:

                def rope_mc(src):
                    mc = wp.tile([128, S], BF16, tag="rope_mc", bufs=1)
                    nc.vector.tensor_mul(mc[:], src[:], cos_t[:])
                    return mc

                def rope_rot(dst, src, mc, tag):
                    for rh in range(2):
                        rp = pp.tile([128, 512], F32, tag="pden", bufs=1,
                                     name=f"rot_{tag}_{rh}")
                        nc.tensor.matmul(
                            rp[:], lhsT=rotm_t[:],
                            rhs=src[:, rh * 512:(rh + 1) * 512],
                            start=True, stop=True,
                        )
                        ms = wp.tile([128, 512], BF16, tag="rope_ms", bufs=1)
                        nc.vector.tensor_mul(ms[:], rp[:], sin_t[:, rh * 512:(rh + 1) * 512])
                        nc.vector.tensor_add(
                            dst[:, rh * 512:(rh + 1) * 512],
                            mc[:, rh * 512:(rh + 1) * 512], ms[:],
                        )

                def rope(dst, src, tag):
                    # dst = src*cos + rotate_half(src)*sin; rotate via PE
                    rope_rot(dst, src, rope_mc(src), tag)

                def rope_half(dst, src, rh, tag):
                    # one token-half of rope: dst[:, rh*512:] from src half
                    sl = slice(rh * 512, (rh + 1) * 512)
                    mch = wp.tile([128, 512], BF16, tag="rope_mc", bufs=1,
                                  name=f"mch_{tag}_{rh}")
                    nc.vector.tensor_mul(mch[:], src[:, sl], cos_t[:, sl])
                    rp = pp.tile([128, 512], F32, tag="pden", bufs=1,
                                 name=f"rot_{tag}_{rh}")
                    nc.tensor.matmul(rp[:], lhsT=rotm_t[:], rhs=src[:, sl],
                                     start=True, stop=True)
                    ms = wp.tile([128, 512], BF16, tag="rope_ms", bufs=1,
                                 name=f"msh_{tag}_{rh}")
                    nc.vector.tensor_mul(ms[:], rp[:], sin_t[:, sl])
                    nc.vector.tensor_add(dst[:, sl], mch[:], ms[:])

                # ---- phase 1: interleaved K / VT / Q-head0 projections ----
                # DMA order matters: the first matmuls need the weights and
                # x chunk 0, so those go first; bulk x and rope consts after.
                wk_t = wp.tile([128, HD], BF16, tag="wq", bufs=3, name="wk")
                wv_t = wp.tile([128, HD], BF16, tag="wq", bufs=3, name="wv")
                wq0_t = wp.tile([128, HD], BF16, tag="wq", bufs=3, name="wq0")
                def wpiece(p):
                    c0 = p * 1024
                    nc.sync.dma_start(out=wk_t[:, c0:c0 + 1024],
                                      in_=wk_d[:, c0:c0 + 1024])
                    nc.sync.dma_start(out=wv_t[:, c0:c0 + 1024],
                                      in_=wv_d[:, c0:c0 + 1024])
                    nc.sync.dma_start(out=wq0_t[:, c0:c0 + 1024],
                                      in_=wq_d[0, :, c0:c0 + 1024])

                # DMA packets drain the queue ~in order, so emit transfers in
                # exact consumption order: weight piece p just before the x
                # chunks that use it.
                for nm, wt_, wd_ in (("wk", None, None),):
                    pass
                nc.sync.dma_start(out=wk_t[:, 0:256], in_=wk_d[:, 0:256])
                nc.sync.dma_start(out=wv_t[:, 0:256], in_=wv_d[:, 0:256])
                nc.sync.dma_start(out=wq0_t[:, 0:256], in_=wq_d[0, :, 0:256])
                for i in range(NI):
                    if i == 1:
                        nc.sync.dma_start(out=wk_t[:, 256:1024],
                                          in_=wk_d[:, 256:1024])
                        nc.sync.dma_start(out=wv_t[:, 256:1024],
                                          in_=wv_d[:, 256:1024])
                        nc.sync.dma_start(out=wq0_t[:, 256:1024],
                                          in_=wq_d[0, :, 256:1024])
                    nc.sync.dma_start(
                        out=xt[:, i * S:(i + 1) * S], in_=xT_d[i * 128:(i + 1) * 128, :]
                    )
                    if i == 5:
                        wpiece(1)
                    if i == 13:
                        wpiece(2)
                    if i == 21:
                        wpiece(3)
                    if i == 8:
                        nc.sync.dma_start(out=cos_t[:], in_=cos_d[:])
                        nc.sync.dma_start(out=sin_t[:], in_=sin_d[:])
                    if i == 10:
                        nc.sync.dma_start(out=alpha_t[:], in_=alpha_d[:])
                        nc.sync.dma_start(out=rotm_t[:], in_=rotm_d[:])
                        nc.sync.dma_start(out=iden_t[:], in_=iden_d[:])
                kps = [pp.tile([128, 512], F32, tag="pscore", bufs=2, name=f"kp{hh}") for hh in range(2)]
                vps = [pp.tile([128, 512], F32, tag="pav", bufs=3, name=f"vp{hh}") for hh in range(2)]
                qps0 = [pp.tile([128, 512], F32, tag="pproj", bufs=2, name=f"qp0{hh}") for hh in range(2)]
                for i in range(NI):
                    st_ = (i == 0)
                    sp_ = (i == NI - 1)
                    for hh in range(2):
                        rhs = xt[:, i * S + hh * 512: i * S + (hh + 1) * 512]
                        nc.tensor.matmul(kps[hh][:], lhsT=wk_t[:, i * 128:(i + 1) * 128],
                                         rhs=rhs, start=st_, stop=sp_)
                        nc.tensor.matmul(vps[hh][:], lhsT=wv_t[:, i * 128:(i + 1) * 128],
                                         rhs=rhs, start=st_, stop=sp_)
                        nc.tensor.matmul(qps0[hh][:], lhsT=wq0_t[:, i * 128:(i + 1) * 128],
                                         rhs=rhs, start=st_, stop=sp_)
                kraw = wp.tile([128, S], BF16, tag="yoA", bufs=2, name="kraw")
                vtraw = wp.tile([128, S], BF16, tag="qrt", bufs=2)
                qraw0 = wp.tile([128, S], BF16, tag="qraw", bufs=1)
                for hh in range(2):
                    nc.scalar.copy(kraw[:, hh * 512:(hh + 1) * 512], kps[hh][:])
                    nc.scalar.copy(qraw0[:, hh * 512:(hh + 1) * 512], qps0[hh][:])
                    nc.vector.tensor_copy(vtraw[:, hh * 512:(hh + 1) * 512], vps[hh][:])
                rope(kr_t, kraw, "k")
                for t in range(NT):
                    tp = pp.tile([128, 128], BF16, tag="pscore", bufs=2, name=f"vtp{t}")
                    nc.tensor.transpose(tp[:], vtraw[:, t * 128:(t + 1) * 128], iden_t[:])
                    nc.scalar.copy(v_all[:, t * 128:(t + 1) * 128], tp[:])
                qrt0 = wp.tile([128, S], BF16, tag="qrt", bufs=2)
                rope(qrt0, qraw0, "q0")

                # ---- phase 2: per-head attention + interleaved Q proj(h+1) ----
                def acol(kc, q):
                    return A_OFF[kc] + q - A_QLO[kc]

                qrt_holder = [qrt0]
                wot_pre = {}
                yps0 = None
                for h in range(H):
                    qrt = qrt_holder[0]

                    # Q projection emitters for head h+1, in 8 groups of 4 i's
                    if h + 1 < H:
                        wq_t = wp.tile([128, HD], BF16, tag="wq", bufs=3)
                        nc.sync.dma_start(out=wq_t[:], in_=wq_d[h + 1])
                        qps = [pp.tile([128, 512], F32, tag="pproj", bufs=2,
                                       name=f"qp{h+1}_{hh}") for hh in range(2)]

                        QG = [(0, 0, 8), (0, 8, 16), (0, 16, 24), (0, 24, 32),
                              (1, 0, 8), (1, 8, 16), (1, 16, 24), (1, 24, 32)]

                        def qgroup(g, wq_t=wq_t, qps=qps):
                            hh, ilo, ihi = QG[g]
                            for i in range(ilo, ihi):
                                nc.tensor.matmul(
                                    qps[hh][:],
                                    lhsT=wq_t[:, i * 128:(i + 1) * 128],
                                    rhs=xt[:, i * S + hh * 512: i * S + (hh + 1) * 512],
                                    start=(i == 0), stop=(i == NI - 1),
                                )
                    else:
                        # last head: no Q projection to interleave — fill the
                        # PE with O-proj tile 0 (heads 0..30 partial sums) in
                        # the idle pproj PSUM banks instead.
                        QG = [(0, 0, 8), (0, 8, 16), (0, 16, 24), (0, 24, 32),
                              (1, 0, 8), (1, 8, 16), (1, 16, 24), (1, 24, 32)]
                        w0t = wot_pre[0]
                        yps0 = [pp.tile([128, 512], F32, tag="pproj", bufs=2,
                                        name=f"y0_{hh}") for hh in range(2)]

                        def qgroup(g, w0t=w0t, yps0=yps0):
                            hh, ilo, ihi = QG[g]
                            for i in range(ilo, min(ihi, 31)):
                                nc.tensor.matmul(
                                    yps0[hh][:],
                                    lhsT=w0t[:, i * 128:(i + 1) * 128],
                                    rhs=ao[:, i * S + hh * 512: i * S + (hh + 1) * 512],
                                    start=(i == 0), stop=False,
                                )

                    if h == H - 2:
                        # prefetch the first O-proj weight slabs (allocated
                        # after wq31 so the wq-slot rotation stays acyclic);
                        # their DMAs overlap the last two heads' attention
                        for ot in range(3):
                            w_pre = wp.tile([128, HD], BF16, tag="wq", bufs=3,
                                            name=f"wo{ot}")
                            nc.sync.dma_start(out=w_pre[:], in_=wo_d[ot])
                            wot_pre[ot] = w_pre

                    atile = wp.tile([128, A_TOT], BF16, tag="attn", bufs=2)

                    def score(kc, qlo, qhi, tag):
                        L = qhi - qlo
                        sp = pp.tile([128, L], F32, tag="pscore", bufs=2,
                                     name=f"sc{h}_{tag}")
                        nc.tensor.matmul(
                            sp[:], lhsT=kr_t[:, kc * 128:(kc + 1) * 128],
                            rhs=qrt[:, qlo:qhi], start=True, stop=True,
                        )
                        nc.scalar.activation(
                            atile[:, acol(kc, qlo):acol(kc, qhi)], sp[:],
                            AF.Sigmoid, scale=SCALE,
                        )

                    def av(dst, kcs, qlo, qhi):
                        # dst += sum_kc V[kc].T @ a[kc, qlo:qhi]
                        for j, kc in enumerate(kcs):
                            nc.tensor.matmul(
                                dst[:],
                                lhsT=v_all[:, kc * 128:(kc + 1) * 128],
                                rhs=atile[:, acol(kc, qlo):acol(kc, qhi)],
                                start=(j == 0), stop=(j == len(kcs) - 1),
                            )

                    def denom(w, kcs, qlo, qhi):
                        # rb = 1 / sum_k sigmoid over window (broadcast on 128 parts)
                        L = qhi - qlo
                        asum = wp.tile([128, L], BF16, tag="asum", bufs=1,
                                       name=f"as{h}_{w}")
                        nc.vector.tensor_add(
                            asum[:], atile[:, acol(kcs[0], qlo):acol(kcs[0], qhi)],
                            atile[:, acol(kcs[1], qlo):acol(kcs[1], qhi)])
                        for kc in kcs[2:]:
                            nc.vector.tensor_add(
                                asum[:], asum[:],
                                atile[:, acol(kc, qlo):acol(kc, qhi)])
                        dn = pp.tile([128, L], F32, tag="pden", bufs=1,
                                     name=f"dn{h}_{w}")
                        nc.tensor.matmul(dn[:], lhsT=ones_t[:], rhs=asum[:],
                                         start=True, stop=True)
                        rb = wp.tile([128, L], F32, tag="rb", bufs=2,
                                     name=f"rb{h}_{w}")
                        nc.vector.reciprocal_approx_fast(rb[:], dn[:])
                        return rb

                    _ncnt = [0]

                    def ntmp(L, tag):
                        _ncnt[0] += 1
                        return wp.tile([128, L], BF16, tag="nt", bufs=4,
                                       name=f"nt{h}_{_ncnt[0]}")

                    def blend(qlo, n_new):
                        # ao[qlo:qlo+256] += alpha * (n_new - ao[qlo:qlo+256])
                        sl = aoh[:, qlo:qlo + 256]
                        _ncnt[0] += 1
                        t2 = wp.tile([128, 256], BF16, tag="bl2", bufs=1,
                                     name=f"bl2_{h}_{_ncnt[0]}")
                        nc.vector.tensor_sub(t2[:], n_new[:], sl)
                        nc.vector.tensor_mul(t2[:], t2[:], alpha_t[:])
                        nc.vector.tensor_add(sl, sl, t2[:])

                    aoh = ao[:, h * S:(h + 1) * S]

                    # --- window 0 ---
                    score(0, 0, 512, "a0")
                    score(1, 0, 512, "a1")
                    qgroup(0)
                    score(2, 0, 512, "a2")
                    score(3, 0, 512, "a3")
                    qgroup(1)
                    pavA = pp.tile([128, 512], F32, tag="pav", bufs=3, name=f"A{h}")
                    av(pavA, [0, 1], 0, 512)
                    pavB = pp.tile([128, 512], F32, tag="pav", bufs=3, name=f"B{h}")
                    av(pavB, [2, 3], 0, 512)
                    rb0 = denom(0, [0, 1, 2, 3], 0, 512)
                    nA = ntmp(512, "n1")
                    nc.vector.tensor_mul(nA[:], pavA[:], rb0[:])
                    nB = ntmp(512, "n2")
                    nc.vector.tensor_mul(nB[:], pavB[:], rb0[:])
                    nc.vector.tensor_add(aoh[:, 0:512], nA[:], nB[:])
                    qgroup(2)

                    # --- window 1 ---
                    score(4, 256, 768, "a4")
                    score(5, 256, 768, "a5")
                    qgroup(3)
                    score(2, 512, 768, "a2b")
                    score(3, 512, 768, "a3b")
                    if h + 1 < H:
                        # token-half 0 of Q(h+1) is done (qgroup(3) stop):
                        # evacuate and rope it now so next head's window-0
                        # scores never wait
                        qraw = wp.tile([128, S], BF16, tag="qraw", bufs=1,
                                       name=f"qraw{h+1}")
                        nc.scalar.copy(qraw[:, 0:512], qps[0][:])
                        qrt_next = wp.tile([128, S], BF16, tag="qrt", bufs=2,
                                           name=f"qrt{h+1}")
                    pavC1 = pp.tile([128, 512], F32, tag="pav", bufs=3, name=f"C1{h}")
                    av(pavC1, [4, 5], 256, 768)
                    pavC2 = pp.tile([128, 256], F32, tag="pav", bufs=3, name=f"C2{h}")
                    av(pavC2, [2, 3], 512, 768)
                    rb1 = denom(1, [2, 3, 4, 5], 256, 768)
                    if h + 1 < H:
                        rope_half(qrt_next, qraw, 0, f"q{h+1}")
                    nB1 = ntmp(256, "n1")
                    nc.vector.tensor_mul(nB1[:], pavB[:, 256:512], rb1[:, 0:256])
                    nC1 = ntmp(512, "n2")
                    nc.vector.tensor_mul(nC1[:], pavC1[:], rb1[:])
                    n1a = ntmp(256, "n3")
                    nc.vector.tensor_add(n1a[:], nB1[:], nC1[:, 0:256])
                    blend(256, n1a)
                    nC2 = ntmp(256, "n1")
                    nc.vector.tensor_mul(nC2[:], pavC2[:], rb1[:, 256:512])
                    nc.vector.tensor_add(aoh[:, 512:768], nC2[:], nC1[:, 256:512])
                    qgroup(4)

                    # --- window 2 --- (F allocated before D so the pav
                    # slot the next head's early groups rotate into frees
                    # at w2-end rather than at w3-end)
                    score(6, 512, 1024, "a6")
                    score(7, 512, 1024, "a7")
                    qgroup(5)
                    score(4, 768, 1024, "a4b")
                    score(5, 768, 1024, "a5b")
                    qgroup(6)
                    pavF = pp.tile([128, 256], F32, tag="pav", bufs=3, name=f"F{h}")
                    av(pavF, [4, 5], 768, 1024)
                    rb2 = denom(2, [4, 5, 6, 7], 512, 1024)
                    nC1b = ntmp(256, "n1")
                    nc.vector.tensor_mul(nC1b[:], pavC1[:, 256:512], rb2[:, 0:256])
                    # window-3 denominator sum now (slot freed by dn2 matmul;
                    # keeps the late den3 matmul off the DVE critical path)
                    asum3 = wp.tile([128, 256], BF16, tag="asum", bufs=1,
                                    name=f"as{h}_3")
                    nc.vector.tensor_add(
                        asum3[:], atile[:, acol(6, 768):acol(6, 1024)],
                        atile[:, acol(7, 768):acol(7, 1024)])
                    qgroup(7)
                    if h + 1 < H:
                        nc.scalar.copy(qraw[:, 512:1024], qps[1][:])
                    pavD = pp.tile([128, 512], F32, tag="pav", bufs=3, name=f"D{h}")
                    av(pavD, [6, 7], 512, 1024)
                    if h + 1 < H:
                        rope_half(qrt_next, qraw, 1, f"q{h+1}")
                        qrt_holder[0] = qrt_next
                    nD = ntmp(512, "n2")
                    nc.vector.tensor_mul(nD[:], pavD[:], rb2[:])
                    n2a = ntmp(256, "n3")
                    nc.vector.tensor_add(n2a[:], nC1b[:], nD[:, 0:256])
                    blend(512, n2a)
                    nF = ntmp(256, "n1")
                    nc.vector.tensor_mul(nF[:], pavF[:], rb2[:, 256:512])
                    nc.vector.tensor_add(aoh[:, 768:1024], nD[:, 256:512], nF[:])

                    # --- window 3 ---
                    dn3 = pp.tile([128, 256], F32, tag="pden", bufs=1,
                                  name=f"dn{h}_3")
                    nc.tensor.matmul(dn3[:], lhsT=ones_t[:], rhs=asum3[:],
                                     start=True, stop=True)
                    rb3 = wp.tile([128, 256], F32, tag="rb", bufs=2,
                                  name=f"rb{h}_3")
                    nc.vector.reciprocal_approx_fast(rb3[:], dn3[:])
                    n3 = ntmp(256, "n2")
                    nc.vector.tensor_mul(n3[:], pavD[:, 256:512], rb3[:])
                    blend(768, n3)

                    if h == H - 1:
                        # close out O-proj tile 0 with the head-31 term
                        for hh in range(2):
                            nc.tensor.matmul(
                                yps0[hh][:],
                                lhsT=wot_pre[0][:, 31 * 128:32 * 128],
                                rhs=ao[:, 31 * S + hh * 512: 31 * S + (hh + 1) * 512],
                                start=False, stop=True,
                            )
                        yoA0 = wp.tile([128, 512], F32, tag="yoA", bufs=2,
                                       name="yoA_t0")
                        nc.scalar.copy(yoA0[:], yps0[0][:])
                        nc.sync.dma_start(out=y_d[0:128, 0:512], in_=yoA0[:])
                        yoB0 = wp.tile([128, 512], F32, tag="yoB", bufs=2,
                                       name="yoB_t0")
                        nc.vector.tensor_copy(yoB0[:], yps0[1][:])
                        nc.sync.dma_start(out=y_d[0:128, 512:1024], in_=yoB0[:])
                        # O-proj tile 1 as one dense stream in the freed
                        # pscore banks — keeps the PE busy through the psum
                        # pool transition
                        w1t = wot_pre[1]
                        yps1 = [pp.tile([128, 512], F32, tag="pscore", bufs=2,
                                        name=f"y1_{hh}") for hh in range(2)]
                        for i in range(NI):
                            for hh in range(2):
                                nc.tensor.matmul(
                                    yps1[hh][:],
                                    lhsT=w1t[:, i * 128:(i + 1) * 128],
                                    rhs=ao[:, i * S + hh * 512: i * S + (hh + 1) * 512],
                                    start=(i == 0), stop=(i == NI - 1),
                                )
                        yoA1 = wp.tile([128, 512], F32, tag="yoA", bufs=2,
                                       name="yoA_t1")
                        nc.scalar.copy(yoA1[:], yps1[0][:])
                        nc.sync.dma_start(out=y_d[128:256, 0:512], in_=yoA1[:])
                        yoB1 = wp.tile([128, 512], F32, tag="yoB", bufs=2,
                                       name="yoB_t1")
                        nc.vector.tensor_copy(yoB1[:], yps1[1][:])
                        nc.sync.dma_start(out=y_d[128:256, 512:1024], in_=yoB1[:])

              # ---- phase 3: O-projection, transposed (yT = Wo @ aoT) ----
              # psum pool `pp` is closed; open a fresh one for y tiles.
              with tc.tile_pool(name="ops", bufs=1, space="PSUM") as opp:
                    for ot in range(2, 32):
                        if ot in wot_pre:
                            wot = wot_pre[ot]
                        else:
                            wot = wp.tile([128, HD], BF16, tag="wq", bufs=3,
                                          name=f"wo{ot}")
                            nc.sync.dma_start(out=wot[:], in_=wo_d[ot])
                        yps = [opp.tile([128, 512], F32, tag="yps", bufs=6,
                                        name=f"yp{ot}_{hh}") for hh in range(2)]
                        for i in range(NI):
                            for hh in range(2):
                                nc.tensor.matmul(
                                    yps[hh][:],
                                    lhsT=wot[:, i * 128:(i + 1) * 128],
                                    rhs=ao[:, i * S + hh * 512: i * S + (hh + 1) * 512],
                                    start=(i == 0), stop=(i == NI - 1),
                                )
                        yoA = wp.tile([128, 512], F32, tag="yoA", bufs=2,
                                      name=f"yoA{ot}")
                        nc.scalar.copy(yoA[:], yps[0][:])
                        nc.sync.dma_start(
                            out=y_d[ot * 128:(ot + 1) * 128, 0:512], in_=yoA[:])
                        yoB = wp.tile([128, 512], F32, tag="yoB", bufs=2,
                                      name=f"yoB{ot}")
                        nc.vector.tensor_copy(yoB[:], yps[1][:])
                        nc.sync.dma_start(
                            out=y_d[ot * 128:(ot + 1) * 128, 512:1024], in_=yoB[:])
    nc.compile()
    return nc


def prep_inputs(x, Wq, Wk, Wv, Wo):
    """Host-side: transpose/tile/cast so every device DMA is contiguous."""
    bf = ml_dtypes.bfloat16
    xT = np.ascontiguousarray(np.transpose(x, (0, 2, 1))).astype(bf)   # [B,4096,1024]
    # wq[h,p,i*128+c] = Wq[h*128+c, i*128+p]
    wq = np.ascontiguousarray(
        Wq.reshape(H, 128, NI, 128).transpose(0, 3, 2, 1).reshape(H, 128, HD)
    ).astype(bf)
    # wk[p, i*128+c] = Wk[c, i*128+p]
    wk = np.ascontiguousarray(
        Wk.reshape(128, NI, 128).transpose(2, 1, 0).reshape(128, HD)
    ).astype(bf)
    wv = np.ascontiguousarray(
        Wv.reshape(128, NI, 128).transpose(2, 1, 0).reshape(128, HD)
    ).astype(bf)
    # wo[ot, p, i*128+c] = Wo[ot*128+c, i*128+p]
    wo = np.ascontiguousarray(
        Wo.reshape(32, 128, NI, 128).transpose(0, 3, 2, 1).reshape(32, 128, HD)
    ).astype(bf)
    cos, sin = _rope_cache_np(S, DH)
    cosT = np.ascontiguousarray(cos.T).astype(bf)                      # [128,1024]
    sinS = np.ascontiguousarray(sin.T).astype(bf)
    rotm = np.zeros((128, 128), dtype=np.float32)
    rotm[np.arange(64) + 64, np.arange(64)] = -1.0
    rotm[np.arange(64), np.arange(64) + 64] = 1.0
    rotm = rotm.astype(bf)
    alphaB = np.tile(
        np.linspace(0.0, 1.0, 256, dtype=np.float32)[None, :], (128, 1)
    ).astype(bf)
    ident = np.eye(128, dtype=np.float32).astype(bf)
    shared = dict(wq=wq, wk=wk, wv=wv, wo=wo, cosT=cosT, sinS=sinS, alphaB=alphaB,
                  rotm=rotm, ident=ident)
    in_maps = [dict(xT=xT[b], **shared) for b in range(B)]
    return in_maps


def kernel(x, Wq, Wk, Wv, Wo):
    if "nc" not in _CACHE:
        _CACHE["nc"] = build_nc()
    nc = _CACHE["nc"]
    in_maps = prep_inputs(
        np.asarray(x, dtype=np.float32),
        np.asarray(Wq, dtype=np.float32),
        np.asarray(Wk, dtype=np.float32),
        np.asarray(Wv, dtype=np.float32),
        np.asarray(Wo, dtype=np.float32),
    )
    res = run_bass_kernel_spmd(nc, in_maps, core_ids=list(range(B)))
    out = np.stack(
        [np.ascontiguousarray(np.asarray(res.results[b]["y"]).T) for b in range(B)],
        axis=0,
    )
    return out.astype(np.float32)


if __name__ == "__main__":
    rng = np.random.default_rng(0)
    x = rng.standard_normal((B, S, HD), dtype=np.float32)
    Wq = (rng.standard_normal((HD, HD), dtype=np.float32) * 0.02)
    Wk = (rng.standard_normal((DH, HD), dtype=np.float32) * 0.02)
    Wv = (rng.standard_normal((DH, HD), dtype=np.float32) * 0.02)
    Wo = (rng.standard_normal((HD, HD), dtype=np.float32) * 0.02)
    y = kernel(x=x, Wq=Wq, Wk=Wk, Wv=Wv, Wo=Wo)
    print("out", y.shape, y.dtype, float(np.abs(y).mean()))


# revision 9
# speedup vs baseline: 1.2598x; 1.0017x over previous
"""Trainium2 Bass kernel for nn_AdvancedFastMQA — v2.

Data-parallel over batch B=8 across 8 NeuronCores. Transposed dataflow
(no on-device transposes except V). Per-core PE-cycle cuts vs v1:

 - Sliding-window overlap sharing: score tiles and attention@V partial
   sums for the k-chunk regions shared by adjacent windows are computed
   once (40 unit tiles instead of 52 for both scores and AV). Window
   outputs are assembled from 2-chunk PSUM partials (A,B,C1,C2,D,F) with
   cheap DVE combines.
 - Denominator: instead of M=1 ones-vector matmuls per k-chunk (same
   streaming cost as scores, zero useful flops), the k-chunk sigmoid
   tiles are summed on DVE and a single ones[128x128] matmul per window
   produces the partition-sum already broadcast across 128 partitions
   (also kills the gpsimd broadcast).
 - O-projection computed transposed: yT[o,t] = sum_i wo_tile[i].T @ ao_i
   with the weight stationary for 2 matmuls each, PSUM double-buffered;
   host transposes the [4096,1024] result back.
 - Q-projection of head h+1 is interleaved into attention of head h so
   the PE never waits on ACT sigmoids.

Windows (S=1024, window=512, stride 256):
  w0: k,q in [0,512); w1: k,q [256,768); w2: k,q [512,1024);
  w3: k,q [768,1024). Blend regions: [256,512) w0/w1, [512,768) w1/w2,
  [768,1024) w2/w3, alpha = linspace(0,1,256).

AV partial-sum plan (per head, PSUM tiles, kc = 128-wide k chunks):
  A  = kc0+kc1 over q[0:512)      B  = kc2+kc3 over q[0:512)
  C1 = kc4+kc5 over q[256:768)    C2 = kc2+kc3 over q[512:768)
  D  = kc6+kc7 over q[512:1024)   F  = kc4+kc5 over q[768:1024)
  u0 = A+B; u1 = B[256:512)+C1 | C2+C1[512:768); u2 = C1+D | D+F;
  u3 = D[768:1024).
"""

import sys

for _p in ("/opt/trn_rl_repo", "/opt/pypackages"):
    if _p not in sys.path:
        sys.path.append(_p)

import numpy as np
import ml_dtypes

import concourse.bacc as bacc
import concourse.tile as tile
import concourse.mybir as mybir
import concourse.bass_isa as bass_isa
from concourse.bass_utils import run_bass_kernel_spmd

BF16 = mybir.dt.bfloat16
F32 = mybir.dt.float32
AF = mybir.ActivationFunctionType

B, S, HD = 8, 1024, 4096
H, DH = 32, 128
WINDOW = 512
SCALE = 1.0 / float(np.sqrt(DH))
ROPE_BASE = 10000.0
NI = HD // 128          # 32 contraction chunks
NT = S // 128           # 8 token chunks

# a-tile (sigmoid) layout: per kc the union of q-ranges that need it.
A_QLO = [0, 0, 0, 0, 256, 256, 512, 512]
A_W = [512, 512, 768, 768, 768, 768, 512, 512]
A_OFF = [0, 512, 1024, 1792, 2560, 3328, 4096, 4608]
A_TOT = 5120

_CACHE = {}


def _rope_cache_np(S_, D_, base=ROPE_BASE):
    inv_freq = 1.0 / (base ** (np.arange(0, D_, 2, dtype=np.float32) / D_))
    t = np.arange(S_, dtype=np.float32)
    f = np.outer(t, inv_freq)
    cos = np.zeros((S_, D_), dtype=np.float32)
    sin = np.zeros((S_, D_), dtype=np.float32)
    cos[:, 0::2] = np.cos(f)
    cos[:, 1::2] = np.cos(f)
    sin[:, 0::2] = np.sin(f)
    sin[:, 1::2] = np.sin(f)
    return cos, sin


def build_nc():
    nc = bacc.Bacc("TRN2", debug=False, target_bir_lowering=False)

    xT_d = nc.dram_tensor("xT", [HD, S], BF16, kind="ExternalInput").ap()
    wq_d = nc.dram_tensor("wq", [H, 128, HD], BF16, kind="ExternalInput").ap()
    wk_d = nc.dram_tensor("wk", [128, HD], BF16, kind="ExternalInput").ap()
    wv_d = nc.dram_tensor("wv", [128, HD], BF16, kind="ExternalInput").ap()
    wo_d = nc.dram_tensor("wo", [32, 128, HD], BF16, kind="ExternalInput").ap()
    cos_d = nc.dram_tensor("cosT", [128, S], BF16, kind="ExternalInput").ap()
    sin_d = nc.dram_tensor("sinS", [128, S], BF16, kind="ExternalInput").ap()
    alpha_d = nc.dram_tensor("alphaB", [128, 256], BF16, kind="ExternalInput").ap()
    rotm_d = nc.dram_tensor("rotm", [128, 128], BF16, kind="ExternalInput").ap()
    iden_d = nc.dram_tensor("ident", [128, 128], BF16, kind="ExternalInput").ap()
    y_d = nc.dram_tensor("y", [HD, S], F32, kind="ExternalOutput").ap()

    with tile.TileContext(nc) as tc:
        with tc.tile_pool(name="consts", bufs=1) as cp:
            xt = cp.tile([128, NI * S], BF16)              # 64KB/part
            cos_t = cp.tile([128, S], BF16)
            sin_t = cp.tile([128, S], BF16)
            alpha_t = cp.tile([128, 256], BF16)
            ones_t = cp.tile([128, 128], BF16)
            nc.vector.memset(ones_t[:], 1.0)
            rotm_t = cp.tile([128, 128], BF16)
            iden_t = cp.tile([128, 128], BF16)

            kr_t = cp.tile([128, S], BF16)                 # roped K
            v_all = cp.tile([128, NT * 128], BF16)         # V as 8 lhsT tiles
            ao = cp.tile([128, H * S], BF16)               # attention out, 64KB/part

            with tc.tile_pool(name="work", bufs=1) as wp:
              with tc.tile_pool(name="ps", bufs=1, space="PSUM") as pp:

                def rope_mc(src):
                    mc = wp.tile([128, S], BF16, tag="rope_mc", bufs=1)
                    nc.vector.tensor_mul(mc[:], src[:], cos_t[:])
                    return mc

                def rope_rot(dst, src, mc, tag):
                    for rh in range(2):
                        rp = pp.tile([128, 512], F32, tag="pden", bufs=1,
                                     name=f"rot_{tag}_{rh}")
                        nc.tensor.matmul(
                            rp[:], lhsT=rotm_t[:],
                            rhs=src[:, rh * 512:(rh + 1) * 512],
                            start=True, stop=True,
                        )
                        ms = wp.tile([128, 512], BF16, tag="rope_ms", bufs=1)
                        nc.vector.tensor_mul(ms[:], rp[:], sin_t[:, rh * 512:(rh + 1) * 512])
                        nc.vector.tensor_add(
                            dst[:, rh * 512:(rh + 1) * 512],
                            mc[:, rh * 512:(rh + 1) * 512], ms[:],
                        )

                def rope(dst, src, tag):
                    # dst = src*cos + rotate_half(src)*sin; rotate via PE
                    rope_rot(dst, src, rope_mc(src), tag)

                def rope_half(dst, src, rh, tag):
                    # one token-half of rope: dst[:, rh*512:] from src half
                    sl = slice(rh * 512, (rh + 1) * 512)
                    mch = wp.tile([128, 512], BF16, tag="rope_mc", bufs=1,
                                  name=f"mch_{tag}_{rh}")
                    nc.vector.tensor_mul(mch[:], src[:, sl], cos_t[:, sl])
                    rp = pp.tile([128, 512], F32, tag="pden", bufs=1,
                                 name=f"rot_{tag}_{rh}")
                    nc.tensor.matmul(rp[:], lhsT=rotm_t[:], rhs=src[:, sl],
                                     start=True, stop=True)
                    ms = wp.tile([128, 512], BF16, tag="rope_ms", bufs=1,
                                 name=f"msh_{tag}_{rh}")
                    nc.vector.tensor_mul(ms[:], rp[:], sin_t[:, sl])
                    nc.vector.tensor_add(dst[:, sl], mch[:], ms[:])

                # ---- phase 1: interleaved K / VT / Q-head0 projections ----
                # DMA order matters: the first matmuls need the weights and
                # x chunk 0, so those go first; bulk x and rope consts after.
                wk_t = wp.tile([128, HD], BF16, tag="wq", bufs=3, name="wk")
                wv_t = wp.tile([128, HD], BF16, tag="wq", bufs=3, name="wv")
                wq0_t = wp.tile([128, HD], BF16, tag="wq", bufs=3, name="wq0")
                def wpiece(p):
                    c0 = p * 1024
                    nc.sync.dma_start(out=wk_t[:, c0:c0 + 1024],
                                      in_=wk_d[:, c0:c0 + 1024])
                    nc.sync.dma_start(out=wv_t[:, c0:c0 + 1024],
                                      in_=wv_d[:, c0:c0 + 1024])
                    nc.sync.dma_start(out=wq0_t[:, c0:c0 + 1024],
                                      in_=wq_d[0, :, c0:c0 + 1024])

                # DMA packets drain the queue ~in order, so emit transfers in
                # exact consumption order: weight piece p just before the x
                # chunks that use it.
                for nm, wt_, wd_ in (("wk", None, None),):
                    pass
                nc.sync.dma_start(out=wk_t[:, 0:256], in_=wk_d[:, 0:256])
                nc.sync.dma_start(out=wv_t[:, 0:256], in_=wv_d[:, 0:256])
                nc.sync.dma_start(out=wq0_t[:, 0:256], in_=wq_d[0, :, 0:256])
                for i in range(NI):
                    if i == 1:
                        nc.sync.dma_start(out=wk_t[:, 256:1024],
                                          in_=wk_d[:, 256:1024])
                        nc.sync.dma_start(out=wv_t[:, 256:1024],
                                          in_=wv_d[:, 256:1024])
                        nc.sync.dma_start(out=wq0_t[:, 256:1024],
                                          in_=wq_d[0, :, 256:1024])
                    nc.sync.dma_start(
                        out=xt[:, i * S:(i + 1) * S], in_=xT_d[i * 128:(i + 1) * 128, :]
                    )
                    for p, base in ((1, 4), (2, 12), (3, 20)):
                        c0 = p * 1024
                        if i == base:
                            nc.sync.dma_start(out=wk_t[:, c0:c0 + 1024],
                                              in_=wk_d[:, c0:c0 + 1024])
                        elif i == base + 1:
                            nc.sync.dma_start(out=wv_t[:, c0:c0 + 1024],
                                              in_=wv_d[:, c0:c0 + 1024])
                        elif i == base + 2:
                            nc.sync.dma_start(out=wq0_t[:, c0:c0 + 1024],
                                              in_=wq_d[0, :, c0:c0 + 1024])
                    if i == 8:
                        nc.sync.dma_start(out=cos_t[:], in_=cos_d[:])
                        nc.sync.dma_start(out=sin_t[:], in_=sin_d[:])
                    if i == 10:
                        nc.sync.dma_start(out=alpha_t[:], in_=alpha_d[:])
                        nc.sync.dma_start(out=rotm_t[:], in_=rotm_d[:])
                        nc.sync.dma_start(out=iden_t[:], in_=iden_d[:])
                kps = [pp.tile([128, 512], F32, tag="pscore", bufs=2, name=f"kp{hh}") for hh in range(2)]
                vps = [pp.tile([128, 512], F32, tag="pav", bufs=3, name=f"vp{hh}") for hh in range(2)]
                qps0 = [pp.tile([128, 512], F32, tag="pproj", bufs=2, name=f"qp0{hh}") for hh in range(2)]
                for i in range(NI):
                    st_ = (i == 0)
                    sp_ = (i == NI - 1)
                    for hh in range(2):
                        rhs = xt[:, i * S + hh * 512: i * S + (hh + 1) * 512]
                        nc.tensor.matmul(kps[hh][:], lhsT=wk_t[:, i * 128:(i + 1) * 128],
                                         rhs=rhs, start=st_, stop=sp_)
                        nc.tensor.matmul(vps[hh][:], lhsT=wv_t[:, i * 128:(i + 1) * 128],
                                         rhs=rhs, start=st_, stop=sp_)
                        nc.tensor.matmul(qps0[hh][:], lhsT=wq0_t[:, i * 128:(i + 1) * 128],
                                         rhs=rhs, start=st_, stop=sp_)
                kraw = wp.tile([128, S], BF16, tag="yoA", bufs=2, name="kraw")
                vtraw = wp.tile([128, S], BF16, tag="qrt", bufs=2)
                qraw0 = wp.tile([128, S], BF16, tag="qraw", bufs=1)
                for hh in range(2):
                    nc.scalar.copy(kraw[:, hh * 512:(hh + 1) * 512], kps[hh][:])
                    nc.scalar.copy(qraw0[:, hh * 512:(hh + 1) * 512], qps0[hh][:])
                    nc.vector.tensor_copy(vtraw[:, hh * 512:(hh + 1) * 512], vps[hh][:])
                rope(kr_t, kraw, "k")
                qrt0 = wp.tile([128, S], BF16, tag="qrt", bufs=2)
                rope_half(qrt0, qraw0, 0, "q0")
                for t in range(NT):
                    tp = pp.tile([128, 128], BF16, tag="pscore", bufs=2, name=f"vtp{t}")
                    nc.tensor.transpose(tp[:], vtraw[:, t * 128:(t + 1) * 128], iden_t[:])
                    nc.scalar.copy(v_all[:, t * 128:(t + 1) * 128], tp[:])
                rope_half(qrt0, qraw0, 1, "q0")

                # ---- phase 2: per-head attention + interleaved Q proj(h+1) ----
                def acol(kc, q):
                    return A_OFF[kc] + q - A_QLO[kc]

                qrt_holder = [qrt0]
                wot_pre = {}
                yps0 = None
                for h in range(H):
                    qrt = qrt_holder[0]

                    # Q projection emitters for head h+1, in 8 groups of 4 i's
                    if h + 1 < H:
                        wq_t = wp.tile([128, HD], BF16, tag="wq", bufs=3)
                        nc.sync.dma_start(out=wq_t[:], in_=wq_d[h + 1])
                        qps = [pp.tile([128, 512], F32, tag="pproj", bufs=2,
                                       name=f"qp{h+1}_{hh}") for hh in range(2)]

                        QG = [(0, 0, 8), (0, 8, 16), (0, 16, 24), (0, 24, 32),
                              (1, 0, 8), (1, 8, 16), (1, 16, 24), (1, 24, 32)]

                        def qgroup(g, wq_t=wq_t, qps=qps):
                            hh, ilo, ihi = QG[g]
                            for i in range(ilo, ihi):
                                nc.tensor.matmul(
                                    qps[hh][:],
                                    lhsT=wq_t[:, i * 128:(i + 1) * 128],
                                    rhs=xt[:, i * S + hh * 512: i * S + (hh + 1) * 512],
                                    start=(i == 0), stop=(i == NI - 1),
                                )
                    else:
                        # last head: no Q projection to interleave — fill the
                        # PE with O-proj tile 0 (heads 0..30 partial sums) in
                        # the idle pproj PSUM banks instead.
                        QG = [(0, 0, 8), (0, 8, 16), (0, 16, 24), (0, 24, 32),
                              (1, 0, 8), (1, 8, 16), (1, 16, 24), (1, 24, 32)]
                        w0t = wot_pre[0]
                        yps0 = [pp.tile([128, 512], F32, tag="pproj", bufs=2,
                                        name=f"y0_{hh}") for hh in range(2)]

                        def qgroup(g, w0t=w0t, yps0=yps0):
                            hh, ilo, ihi = QG[g]
                            for i in range(ilo, min(ihi, 31)):
                                nc.tensor.matmul(
                                    yps0[hh][:],
                                    lhsT=w0t[:, i * 128:(i + 1) * 128],
                                    rhs=ao[:, i * S + hh * 512: i * S + (hh + 1) * 512],
                                    start=(i == 0), stop=False,
                                )

                    if h == H - 2:
                        # prefetch the first O-proj weight slabs (allocated
                        # after wq31 so the wq-slot rotation stays acyclic);
                        # their DMAs overlap the last two heads' attention
                        for ot in range(3):
                            w_pre = wp.tile([128, HD], BF16, tag="wq", bufs=3,
                                            name=f"wo{ot}")
                            nc.sync.dma_start(out=w_pre[:], in_=wo_d[ot])
                            wot_pre[ot] = w_pre

                    atile = wp.tile([128, A_TOT], BF16, tag="attn", bufs=2)

                    def score(kc, qlo, qhi, tag):
                        L = qhi - qlo
                        sp = pp.tile([128, L], F32, tag="pscore", bufs=2,
                                     name=f"sc{h}_{tag}")
                        nc.tensor.matmul(
                            sp[:], lhsT=kr_t[:, kc * 128:(kc + 1) * 128],
                            rhs=qrt[:, qlo:qhi], start=True, stop=True,
                        )
                        nc.scalar.activation(
                            atile[:, acol(kc, qlo):acol(kc, qhi)], sp[:],
                            AF.Sigmoid, scale=SCALE,
                        )

                    def av(dst, kcs, qlo, qhi):
                        # dst += sum_kc V[kc].T @ a[kc, qlo:qhi]
                        for j, kc in enumerate(kcs):
                            nc.tensor.matmul(
                                dst[:],
                                lhsT=v_all[:, kc * 128:(kc + 1) * 128],
                                rhs=atile[:, acol(kc, qlo):acol(kc, qhi)],
                                start=(j == 0), stop=(j == len(kcs) - 1),
                            )

                    def denom(w, kcs, qlo, qhi):
                        # rb = 1 / sum_k sigmoid over window (broadcast on 128 parts)
                        L = qhi - qlo
                        asum = wp.tile([128, L], BF16, tag="asum", bufs=1,
                                       name=f"as{h}_{w}")
                        nc.vector.tensor_add(
                            asum[:], atile[:, acol(kcs[0], qlo):acol(kcs[0], qhi)],
                            atile[:, acol(kcs[1], qlo):acol(kcs[1], qhi)])
                        for kc in kcs[2:]:
                            nc.vector.tensor_add(
                                asum[:], asum[:],
                                atile[:, acol(kc, qlo):acol(kc, qhi)])
                        dn = pp.tile([128, L], F32, tag="pden", bufs=1,
                                     name=f"dn{h}_{w}")
                        nc.tensor.matmul(dn[:], lhsT=ones_t[:], rhs=asum[:],
                                         start=True, stop=True)
                        rb = wp.tile([128, L], F32, tag="rb", bufs=2,
                                     name=f"rb{h}_{w}")
                        nc.vector.reciprocal_approx_fast(rb[:], dn[:])
                        return rb

                    _ncnt = [0]

                    def ntmp(L, tag):
                        _ncnt[0] += 1
                        return wp.tile([128, L], BF16, tag="nt", bufs=4,
                                       name=f"nt{h}_{_ncnt[0]}")

                    def blend(qlo, n_new):
                        # ao[qlo:qlo+256] += alpha * (n_new - ao[qlo:qlo+256])
                        sl = aoh[:, qlo:qlo + 256]
                        _ncnt[0] += 1
                        t2 = wp.tile([128, 256], BF16, tag="bl2", bufs=1,
                                     name=f"bl2_{h}_{_ncnt[0]}")
                        nc.vector.tensor_sub(t2[:], n_new[:], sl)
                        nc.vector.tensor_mul(t2[:], t2[:], alpha_t[:])
                        nc.vector.tensor_add(sl, sl, t2[:])

                    aoh = ao[:, h * S:(h + 1) * S]

                    # --- window 0 ---
                    score(0, 0, 512, "a0")
                    score(1, 0, 512, "a1")
                    qgroup(0)
                    score(2, 0, 512, "a2")
                    score(3, 0, 512, "a3")
                    qgroup(1)
                    pavA = pp.tile([128, 512], F32, tag="pav", bufs=3, name=f"A{h}")
                    av(pavA, [0, 1], 0, 512)
                    pavB = pp.tile([128, 512], F32, tag="pav", bufs=3, name=f"B{h}")
                    av(pavB, [2, 3], 0, 512)
                    rb0 = denom(0, [0, 1, 2, 3], 0, 512)
                    nA = ntmp(512, "n1")
                    nc.vector.tensor_mul(nA[:], pavA[:], rb0[:])
                    nB = ntmp(512, "n2")
                    nc.vector.tensor_mul(nB[:], pavB[:], rb0[:])
                    nc.vector.tensor_add(aoh[:, 0:512], nA[:], nB[:])
                    qgroup(2)

                    # --- window 1 ---
                    score(4, 256, 768, "a4")
                    score(5, 256, 768, "a5")
                    qgroup(3)
                    score(2, 512, 768, "a2b")
                    score(3, 512, 768, "a3b")
                    if h + 1 < H:
                        # token-half 0 of Q(h+1) is done (qgroup(3) stop):
                        # evacuate and rope it now so next head's window-0
                        # scores never wait
                        qraw = wp.tile([128, S], BF16, tag="qraw", bufs=1,
                                       name=f"qraw{h+1}")
                        nc.scalar.copy(qraw[:, 0:512], qps[0][:])
                        qrt_next = wp.tile([128, S], BF16, tag="qrt", bufs=2,
                                           name=f"qrt{h+1}")
                    pavC1 = pp.tile([128, 512], F32, tag="pav", bufs=3, name=f"C1{h}")
                    av(pavC1, [4, 5], 256, 768)
                    pavC2 = pp.tile([128, 256], F32, tag="pav", bufs=3, name=f"C2{h}")
                    av(pavC2, [2, 3], 512, 768)
                    rb1 = denom(1, [2, 3, 4, 5], 256, 768)
                    if h + 1 < H:
                        rope_half(qrt_next, qraw, 0, f"q{h+1}")
                    nB1 = ntmp(256, "n1")
                    nc.vector.tensor_mul(nB1[:], pavB[:, 256:512], rb1[:, 0:256])
                    nC1 = ntmp(512, "n2")
                    nc.vector.tensor_mul(nC1[:], pavC1[:], rb1[:])
                    n1a = ntmp(256, "n3")
                    nc.vector.tensor_add(n1a[:], nB1[:], nC1[:, 0:256])
                    blend(256, n1a)
                    nC2 = ntmp(256, "n1")
                    nc.vector.tensor_mul(nC2[:], pavC2[:], rb1[:, 256:512])
                    nc.vector.tensor_add(aoh[:, 512:768], nC2[:], nC1[:, 256:512])
                    qgroup(4)

                    # --- window 2 --- (F allocated before D so the pav
                    # slot the next head's early groups rotate into frees
                    # at w2-end rather than at w3-end)
                    score(6, 512, 1024, "a6")
                    score(7, 512, 1024, "a7")
                    qgroup(5)
                    score(4, 768, 1024, "a4b")
                    score(5, 768, 1024, "a5b")
                    qgroup(6)
                    pavF = pp.tile([128, 256], F32, tag="pav", bufs=3, name=f"F{h}")
                    av(pavF, [4, 5], 768, 1024)
                    rb2 = denom(2, [4, 5, 6, 7], 512, 1024)
                    nC1b = ntmp(256, "n1")
                    nc.vector.tensor_mul(nC1b[:], pavC1[:, 256:512], rb2[:, 0:256])
                    # window-3 denominator sum now (slot freed by dn2 matmul;
                    # keeps the late den3 matmul off the DVE critical path)
                    asum3 = wp.tile([128, 256], BF16, tag="asum", bufs=1,
                                    name=f"as{h}_3")
                    nc.vector.tensor_add(
                        asum3[:], atile[:, acol(6, 768):acol(6, 1024)],
                        atile[:, acol(7, 768):acol(7, 1024)])
                    qgroup(7)
                    if h + 1 < H:
                        nc.scalar.copy(qraw[:, 512:1024], qps[1][:])
                    pavD = pp.tile([128, 512], F32, tag="pav", bufs=3, name=f"D{h}")
                    av(pavD, [6, 7], 512, 1024)
                    if h + 1 < H:
                        rope_half(qrt_next, qraw, 1, f"q{h+1}")
                        qrt_holder[0] = qrt_next
                    nD = ntmp(512, "n2")
                    nc.vector.tensor_mul(nD[:], pavD[:], rb2[:])
                    n2a = ntmp(256, "n3")
                    nc.vector.tensor_add(n2a[:], nC1b[:], nD[:, 0:256])
                    blend(512, n2a)
                    nF = ntmp(256, "n1")
                    nc.vector.tensor_mul(nF[:], pavF[:], rb2[:, 256:512])
                    nc.vector.tensor_add(aoh[:, 768:1024], nD[:, 256:512], nF[:])

                    # --- window 3 ---
                    dn3 = pp.tile([128, 256], F32, tag="pden", bufs=1,
                                  name=f"dn{h}_3")
                    nc.tensor.matmul(dn3[:], lhsT=ones_t[:], rhs=asum3[:],
                                     start=True, stop=True)
                    rb3 = wp.tile([128, 256], F32, tag="rb", bufs=2,
                                  name=f"rb{h}_3")
                    nc.vector.reciprocal_approx_fast(rb3[:], dn3[:])
                    n3 = ntmp(256, "n2")
                    nc.vector.tensor_mul(n3[:], pavD[:, 256:512], rb3[:])
                    blend(768, n3)

                    if h == H - 1:
                        # close out O-proj tile 0 with the head-31 term
                        for hh in range(2):
                            nc.tensor.matmul(
                                yps0[hh][:],
                                lhsT=wot_pre[0][:, 31 * 128:32 * 128],
                                rhs=ao[:, 31 * S + hh * 512: 31 * S + (hh + 1) * 512],
                                start=False, stop=True,
                            )
                        yoA0 = wp.tile([128, 512], F32, tag="yoA", bufs=2,
                                       name="yoA_t0")
                        nc.scalar.copy(yoA0[:], yps0[0][:])
                        nc.sync.dma_start(out=y_d[0:128, 0:512], in_=yoA0[:])
                        yoB0 = wp.tile([128, 512], F32, tag="yoB", bufs=2,
                                       name="yoB_t0")
                        nc.vector.tensor_copy(yoB0[:], yps0[1][:])
                        nc.sync.dma_start(out=y_d[0:128, 512:1024], in_=yoB0[:])
                        # O-proj tile 1 as one dense stream in the freed
                        # pscore banks — keeps the PE busy through the psum
                        # pool transition
                        w1t = wot_pre[1]
                        yps1 = [pp.tile([128, 512], F32, tag="pscore", bufs=2,
                                        name=f"y1_{hh}") for hh in range(2)]
                        for i in range(NI):
                            for hh in range(2):
                                nc.tensor.matmul(
                                    yps1[hh][:],
                                    lhsT=w1t[:, i * 128:(i + 1) * 128],
                                    rhs=ao[:, i * S + hh * 512: i * S + (hh + 1) * 512],
                                    start=(i == 0), stop=(i == NI - 1),
                                )
                        yoA1 = wp.tile([128, 512], F32, tag="yoA", bufs=2,
                                       name="yoA_t1")
                        nc.scalar.copy(yoA1[:], yps1[0][:])
                        nc.sync.dma_start(out=y_d[128:256, 0:512], in_=yoA1[:])
                        yoB1 = wp.tile([128, 512], F32, tag="yoB", bufs=2,
                                       name="yoB_t1")
                        nc.vector.tensor_copy(yoB1[:], yps1[1][:])
                        nc.sync.dma_start(out=y_d[128:256, 512:1024], in_=yoB1[:])

              # ---- phase 3: O-projection, transposed (yT = Wo @ aoT) ----
              # psum pool `pp` is closed; open a fresh one for y tiles.
              with tc.tile_pool(name="ops", bufs=1, space="PSUM") as opp:
                    for ot in range(2, 32):
                        if ot in wot_pre:
                            wot = wot_pre[ot]
                        else:
                            wot = wp.tile([128, HD], BF16, tag="wq", bufs=3,
                                          name=f"wo{ot}")
                            nc.sync.dma_start(out=wot[:, 0:2048],
                                              in_=wo_d[ot, :, 0:2048])
                            nc.sync.dma_start(out=wot[:, 2048:4096],
                                              in_=wo_d[ot, :, 2048:4096])
                        yps = [opp.tile([128, 512], F32, tag="yps", bufs=8,
                                        name=f"yp{ot}_{hh}") for hh in range(2)]
                        for i in range(NI):
                            for hh in range(2):
                                nc.tensor.matmul(
                                    yps[hh][:],
                                    lhsT=wot[:, i * 128:(i + 1) * 128],
                                    rhs=ao[:, i * S + hh * 512: i * S + (hh + 1) * 512],
                                    start=(i == 0), stop=(i == NI - 1),
                                )
                        yoA = wp.tile([128, 512], F32, tag="yoA", bufs=2,
                                      name=f"yoA{ot}")
                        nc.scalar.copy(yoA[:], yps[0][:])
                        nc.sync.dma_start(
                            out=y_d[ot * 128:(ot + 1) * 128, 0:512], in_=yoA[:])
                        yoB = wp.tile([128, 512], F32, tag="yoB", bufs=2,
                                      name=f"yoB{ot}")
                        nc.vector.tensor_copy(yoB[:], yps[1][:])
                        nc.sync.dma_start(
                            out=y_d[ot * 128:(ot + 1) * 128, 512:1024], in_=yoB[:])
    nc.compile()
    return nc


def prep_inputs(x, Wq, Wk, Wv, Wo):
    """Host-side: transpose/tile/cast so every device DMA is contiguous."""
    bf = ml_dtypes.bfloat16
    xT = np.ascontiguousarray(np.transpose(x, (0, 2, 1))).astype(bf)   # [B,4096,1024]
    # wq[h,p,i*128+c] = Wq[h*128+c, i*128+p]
    wq = np.ascontiguousarray(
        Wq.reshape(H, 128, NI, 128).transpose(0, 3, 2, 1).reshape(H, 128, HD)
    ).astype(bf)
    # wk[p, i*128+c] = Wk[c, i*128+p]
    wk = np.ascontiguousarray(
        Wk.reshape(128, NI, 128).transpose(2, 1, 0).reshape(128, HD)
    ).astype(bf)
    wv = np.ascontiguousarray(
        Wv.reshape(128, NI, 128).transpose(2, 1, 0).reshape(128, HD)
    ).astype(bf)
    # wo[ot, p, i*128+c] = Wo[ot*128+c, i*128+p]
    wo = np.ascontiguousarray(
        Wo.reshape(32, 128, NI, 128).transpose(0, 3, 2, 1).reshape(32, 128, HD)
    ).astype(bf)
    cos, sin = _rope_cache_np(S, DH)
    cosT = np.ascontiguousarray(cos.T).astype(bf)                      # [128,1024]
    sinS = np.ascontiguousarray(sin.T).astype(bf)
    rotm = np.zeros((128, 128), dtype=np.float32)
    rotm[np.arange(64) + 64, np.arange(64)] = -1.0
    rotm[np.arange(64), np.arange(64) + 64] = 1.0
    rotm = rotm.astype(bf)
    alphaB = np.tile(
        np.linspace(0.0, 1.0, 256, dtype=np.float32)[None, :], (128, 1)
    ).astype(bf)
    ident = np.eye(128, dtype=np.float32).astype(bf)
    shared = dict(wq=wq, wk=wk, wv=wv, wo=wo, cosT=cosT, sinS=sinS, alphaB=alphaB,
                  rotm=rotm, ident=ident)
    in_maps = [dict(xT=xT[b], **shared) for b in range(B)]
    return in_maps


def kernel(x, Wq, Wk, Wv, Wo):
    if "nc" not in _CACHE:
        _CACHE["nc"] = build_nc()
    nc = _CACHE["nc"]
    in_maps = prep_inputs(
        np.asarray(x, dtype=np.float32),
        np.asarray(Wq, dtype=np.float32),
        np.asarray(Wk, dtype=np.float32),
        np.asarray(Wv, dtype=np.float32),
        np.asarray(Wo, dtype=np.float32),
    )
    res = run_bass_kernel_spmd(nc, in_maps, core_ids=list(range(B)))
    out = np.stack(
        [np.ascontiguousarray(np.asarray(res.results[b]["y"]).T) for b in range(B)],
        axis=0,
    )
    return out.astype(np.float32)


if __name__ == "__main__":
    rng = np.random.default_rng(0)
    x = rng.standard_normal((B, S, HD), dtype=np.float32)
    Wq = (rng.standard_normal((HD, HD), dtype=np.float32) * 0.02)
    Wk = (rng.standard_normal((DH, HD), dtype=np.float32) * 0.02)
    Wv = (rng.standard_normal((DH, HD), dtype=np.float32) * 0.02)
    Wo = (rng.standard_normal((HD, HD), dtype=np.float32) * 0.02)
    y = kernel(x=x, Wq=Wq, Wk=Wk, Wv=Wv, Wo=Wo)
    print("out", y.shape, y.dtype, float(np.abs(y).mean()))
